# revision 47
# baseline (speedup 1.0000x reference)
"""DenseCL loss kernel for 8 TRN2 NeuronCores (v2: fp8 DoubleRow + column-
sharded dense-InfoNCE logits).

Sharding: core c owns image c (dense head + correspondence + matched keys),
queue rows [c*8192, (c+1)*8192), and the COLUMN shard of the flat dense
logits: core c computes partial exp-sums over its own 784 matched-key
columns for ALL 6272 q rows; the host sums the per-core z partials.  The
only critical-path collective is a single early AllGather of the fp8
q_d (+ q_g) launched right after the q branch, hidden under the k branch.

Dense/global head matmuls run in fp8e4 with DoubleRow (2 contraction rows
per PE cell); weights are pre-scaled x64 on the host, the 1/64 folds into
the activation scale.  End-to-end fp8 rel-err vs the fp32 reference is
~5e-4 (validated in numpy), far under the 2e-2 gate.
"""
import sys

if "/opt/trn_rl_repo" not in sys.path:
    sys.path.insert(0, "/opt/trn_rl_repo")

import numpy as np
import ml_dtypes

import concourse.bass as bass
import concourse.bacc as bacc
import concourse.mybir as mybir
import concourse.tile as tile
from concourse import bass_utils, masks

F8np = ml_dtypes.float8_e4m3     # TRN FP8_EXP4-compatible (bias 7, max 240)
BFnp = ml_dtypes.bfloat16
F32 = mybir.dt.float32
BF16 = mybir.dt.bfloat16
F8 = mybir.dt.float8e4
DR = mybir.MatmulPerfMode.DoubleRow

N_CORES = 8
B, HW, C, D, P, Q = 8, 784, 1024, 2048, 128, 65536
QSH = Q // N_CORES          # 8192 queue rows per core
CT, DT = C // 128, D // 128  # 8, 16
NT = B * HW                 # 6272 total dense rows
RT = NT // 128              # 49 flat q-row tiles
TAU = 0.2
LAM = 0.5
ISC = 1.0 / TAU             # 5.0
AF = mybir.ActivationFunctionType
ALU = mybir.AluOpType

# 784 = 6*128 + 16 partition tiles (correspondence)
PT = [(i * 128, min(128, HW - i * 128)) for i in range(7)]
CH = [(0, 512), (512, HW - 512)]   # free-dim chunks of 784


def _patch_act_tables():
    """Force every activation we use onto the natural_log_exp_and_others
    table set so the kernel needs exactly one ACT_TABLE_LOAD."""
    import concourse.bacc as bacc_mod
    if getattr(bacc_mod, "_act_tables_patched", False):
        return
    from concourse import hw_specs
    orig = hw_specs.get_activation_tables
    ours = {AF.Exp, AF.Ln, AF.Relu, AF.Identity, AF.Copy, AF.Square}
    keep = "natural_log_exp_and_others"

    def patched(arch):
        tabs = orig(arch)
        assert keep in tabs and ours <= tabs[keep]
        return {name: (fns if name == keep else fns - ours)
                for name, fns in tabs.items()}

    bacc_mod.get_activation_tables = patched
    bacc_mod._act_tables_patched = True


def _build(do_compile=True):
    _patch_act_tables()
    nc = bacc.Bacc("TRN2", target_bir_lowering=False, debug=False,
                   num_devices=N_CORES)

    def inp(name, shape, dt):
        return nc.dram_tensor(name, list(shape), dt, kind="ExternalInput")

    env = {}
    env["xq_d"] = inp("xq", (128, CT * HW), F8)    # [c, ct*784+p]
    env["xk_d"] = inp("xk", (128, CT * HW), F8)
    env["wd1_d"] = inp("wd1", (DT // 2, 128, 2 * C), F8)  # dt-pairs, x64
    env["wd1m_d"] = inp("wd1m", (DT // 2, 128, 2 * C), F8)
    env["wd2_d"] = inp("wd2", (128, D), F8)        # [d, dt*128+p] x64
    env["wd2m_d"] = inp("wd2m", (128, D), F8)
    env["wg1_d"] = inp("wg1", (128, CT * D), F8)   # [c, ct*2048+d] x64
    env["wg1m_d"] = inp("wg1m", (128, CT * D), F8)
    env["wg2_d"] = inp("wg2", (128, D), F8)        # like wd2, x64
    env["wg2m_d"] = inp("wg2m", (128, D), F8)
    env["bd1_d"] = inp("bd1", (128, DT), F32)      # [r, dt] = bd1[dt*128+r]
    env["bd1m_d"] = inp("bd1m", (128, DT), F32)
    env["bd2_d"] = inp("bd2", (128, 1), F32)
    env["bd2m_d"] = inp("bd2m", (128, 1), F32)
    env["bg1r_d"] = inp("bg1r", (1, D), BF16)      # bg1 x4096 (bias row)
    env["bg1mr_d"] = inp("bg1mr", (1, D), BF16)
    env["bg2_d"] = inp("bg2", (128, 1), F32)
    env["bg2m_d"] = inp("bg2m", (128, 1), F32)
    env["queueT_d"] = inp("queueT", (128, QSH), F8)  # 8*queue[c0+j, ch]
    env["iota_d"] = inp("iota", (128, 1), F32)
    env["onesc_d"] = inp("onesc", (128, 1), F32)
    env["onesr_d"] = inp("onesr", (1, 128), F32)
    env["ones8_d"] = inp("ones8", (1, 8), BF16)

    env["outz_d"] = nc.dram_tensor("outz", [128, RT], F32,
                                   kind="ExternalOutput")
    env["outs_d"] = nc.dram_tensor("outs", [1, 16], F32,
                                   kind="ExternalOutput")

    with tile.TileContext(nc) as tc:
        with tc.tile_pool(name="dramp", bufs=1, space="DRAM") as dpool:
            env["ag_in"] = dpool.tile([128 * 785], F8, name="ag_in")
            env["ag_out"] = dpool.tile([N_CORES * 128 * 785], F8,
                                       name="ag_out", addr_space="Shared")
            with tc.tile_pool(name="cst", bufs=1) as cst:
                _body(nc, tc, env, cst)
    if do_compile:
        nc.compile()
    return nc


def _dense_branch(nc, tc, env, cst, br, pools, tiles, w1tiles=None):
    """One dense-head branch (q: br=0, k: br=1) -> normalized [128, HW]."""
    g = lambda k: env[k]
    sfx = "" if br == 0 else "m"
    x_sb = tiles["xq8" if br == 0 else "xk8"]
    w1_d = g("wd1" + sfx + "_d")
    w2_sb = tiles["wd2" + sfx]
    b1 = tiles["bd1" + sfx]
    b2 = tiles["bd2" + sfx]
    dst_bf = tiles["qdT_bf" if br == 0 else "kdT_bf"]
    w1p, hp, l2s, ps_h, ps_m = (pools["w1p"], pools["hp"], pools["l2s"],
                                pools["ps_h"], pools["ps_m"])

    xv = x_sb[:].rearrange("c (t p) -> c t p", t=CT)
    qd_ps = ps_m.tile([128, HW], F32, name=f"qd_ps{br}", tag="m")
    hq = None
    hq_prev = None

    def l2_pair(hsrc, dp):
        w2v = w2_sb[:].rearrange("c (t d) -> c t d", t=DT)
        hv = hsrc[:].rearrange("c (j p) -> c j p", j=2)
        for (o, n) in CH:
            nc.tensor.matmul(
                qd_ps[:, o:o + n],
                lhsT=w2v[:, 2 * dp:2 * dp + 2, :],
                rhs=hv[:, :, o:o + n],
                start=(dp == 0), stop=(dp == DT // 2 - 1),
                perf_mode=DR)

    for dt in range(DT):
        # one DMA per dt-pair: 2 KB/partition transfers use the DMA
        # engines far better than 1 KB ones
        if dt % 2 == 0:
            w1t = w1tiles[dt // 2] if w1tiles else w1p.tile(
                [128, 2 * C], F8, name=f"w1t{br}")
            if not w1tiles:
                nc.sync.dma_start(w1t[:], w1_d[dt // 2, :, :])
        h_ps = ps_h.tile([128, HW], F32, name="h_ps", tag="h")
        wv = w1t[:, (dt % 2) * C:(dt % 2 + 1) * C].rearrange(
            "c (t d) -> c t d", t=CT)
        for kp in range(CT // 2):
            for (o, n) in CH:
                nc.tensor.matmul(
                    h_ps[:, o:o + n],
                    lhsT=wv[:, 2 * kp:2 * kp + 2, :],
                    rhs=xv[:, 2 * kp:2 * kp + 2, o:o + n],
                    start=(kp == 0), stop=(kp == CT // 2 - 1),
                    perf_mode=DR)
        # L2 for the pair two dts back: by now its relus have finished, so
        # the PE never stalls waiting on the ACT chain
        if dt % 2 == 0 and dt >= 2:
            l2_pair(hq, dt // 2 - 1)
        if dt % 2 == 0:
            hq = hp.tile([128, 2 * HW], F8, name=f"hq{br}")
        nc.scalar.activation(hq[:, (dt % 2) * HW:(dt % 2 + 1) * HW],
                             h_ps[:], AF.Relu, bias=b1[:, dt:dt + 1],
                             scale=1.0 / 64.0)
        if br == 0:
            # stagger the k-branch / tail input DMAs on the scalar ring so
            # they don't compete with the q-critical loads
            if dt == 2:
                nc.gpsimd.dma_start(tiles["xk8"][:], g("xk_d")[:])
                # pooled sums for the k ghead: must be emitted after the
                # xk8 DMA (program order defines the dependency), runs on
                # the otherwise idle DVE during the q branch
                gsum = tiles["gsum1"]
                for ct in range(CT):
                    pscr = pools["pscr"].tile([128, HW], F8, name="pscr")
                    nc.vector.tensor_scalar(
                        pscr[:], tiles["xk8"][:, ct * HW:(ct + 1) * HW],
                        1.0, None, op0=ALU.mult, op1=ALU.add,
                        accum_out=gsum[:, ct:ct + 1])

            elif dt == 4:
                nc.scalar.dma_start(tiles["wg1"][:], g("wg1_d")[:])
                nc.scalar.dma_start(tiles["wg2"][:], g("wg2_d")[:])
            elif dt == 8:
                nc.scalar.dma_start(tiles["wd2m"][:], g("wd2m_d")[:])

    l2_pair(hq, DT // 2 - 1)

    # bias + l2 normalize along channels (partition dim)
    qdT_f = l2s.tile([128, HW], F32, name=f"qdT_f{br}")
    nc.scalar.activation(qdT_f[:], qd_ps[:], AF.Identity, bias=b2[:],
                         scale=1.0 / 64.0)
    sq = l2s.tile([128, HW], BF16, name=f"sq{br}")
    nc.scalar.activation(sq[:], qdT_f[:], AF.Square)
    ssq_ps = ps_m.tile([1, HW], F32, name=f"ssq{br}", tag="m")
    for (o, n) in CH:
        nc.tensor.matmul(ssq_ps[:, o:o + n], lhsT=tiles["onescb"][:],
                         rhs=sq[:, o:o + n], start=True, stop=True)
    nrm = l2s.tile([1, HW], F32, name=f"nrm{br}")
    nc.vector.tensor_scalar_max(nrm[:], ssq_ps[:], 1e-12)
    nrm2 = l2s.tile([1, HW], F32, name=f"nrm2{br}")
    nc.scalar.activation(nrm2[:], nrm[:], AF.Ln)
    rn = l2s.tile([1, HW], F32, name=f"rn{br}")
    nc.scalar.activation(rn[:], nrm2[:], AF.Exp, scale=-0.5)
    rnb_ps = ps_m.tile([128, HW], F32, name=f"rnb{br}", tag="m")
    for (o, n) in CH:
        nc.tensor.matmul(rnb_ps[:, o:o + n], lhsT=tiles["onesr"][:],
                         rhs=rn[:, o:o + n], start=True, stop=True)
    nc.vector.tensor_mul(dst_bf[:], qdT_f[:], rnb_ps[:])
    return dst_bf


def _ghead_branch(nc, tc, env, cst, br, pools, tiles):
    """Global head for the core's own image (q: br=0, k: br=1)."""
    g = lambda k: env[k]
    sfx = "" if br == 0 else "m"
    x_sb = tiles["xq8" if br == 0 else "xk8"]
    w1_sb = tiles["wg1" + sfx]
    w2_sb = tiles["wg2" + sfx]
    b1r = tiles["bg1r" if br == 0 else "bg1mr"]
    b2 = tiles["bg2" + sfx]
    dst_bf = tiles["qgT_bf" if br == 0 else "kgT_bf"]
    gp, ps_m = pools["gp"], pools["ps_m"]
    ones1 = tiles["ones8"][0:1, 0:1]

    # pooled sums were computed up front on the DVE; scale to g*64 staged
    # at stride 16 for the DoubleRow stationary
    gsum = tiles[f"gsum{br}"]
    gqt8 = gp.tile([128, CT * 16], F8, name=f"gqt8{br}")
    gq_v = gqt8[:].rearrange("c (t s) -> c t s", s=16)
    nc.vector.tensor_scalar_mul(gq_v[:, :, 0:1], gsum[:], 64.0 / HW)

    # L1: h_g[1, 2048] = (g*64) @ (Wg1*64) / 4096 + bg1, in 512-chunks
    hgb = gp.tile([1, D], BF16, name=f"hgb{br}")
    w1v = w1_sb[:].rearrange("c (t d) -> c t d", t=CT)
    for ch in range(4):
        hg_ps = ps_m.tile([1, 512], F32, name=f"hg{br}", tag="m")
        for kp in range(CT // 2):
            nc.tensor.matmul(
                hg_ps[:], lhsT=gq_v[:, 2 * kp:2 * kp + 2, 0:1],
                rhs=w1v[:, 2 * kp:2 * kp + 2, ch * 512:(ch + 1) * 512],
                start=(kp == 0), stop=False, perf_mode=DR)
        nc.tensor.matmul(hg_ps[:], lhsT=ones1,
                         rhs=b1r[0:1, ch * 512:(ch + 1) * 512],
                         start=False, stop=True)
        nc.scalar.activation(hgb[0:1, ch * 512:(ch + 1) * 512], hg_ps[:],
                             AF.Relu, scale=1.0 / 4096.0)
    # transpose h_g -> [128, DT] via K=1 matmuls, then fp8 at stride 16
    hgt_ps = ps_m.tile([128, DT], F32, name=f"hgt{br}", tag="m")
    for dt in range(DT):
        nc.tensor.matmul(hgt_ps[:, dt:dt + 1],
                         lhsT=hgb[0:1, dt * 128:(dt + 1) * 128],
                         rhs=ones1, start=(dt == 0), stop=(dt == DT - 1))
    hgt8 = gp.tile([128, DT * 16], F8, name=f"hgt8{br}")
    hgt_v = hgt8[:].rearrange("c (t s) -> c t s", s=16)
    nc.scalar.activation(hgt_v[:, :, 0:1], hgt_ps[:], AF.Copy)
    # L2: q_g[128, 1]
    qg_ps = ps_m.tile([128, 1], F32, name=f"qg{br}", tag="m")
    w2v = w2_sb[:].rearrange("c (t d) -> c t d", t=DT)
    for dp in range(DT // 2):
        nc.tensor.matmul(qg_ps[:], lhsT=w2v[:, 2 * dp:2 * dp + 2, :],
                         rhs=hgt_v[:, 2 * dp:2 * dp + 2, 0:1],
                         start=(dp == 0), stop=(dp == DT // 2 - 1),
                         perf_mode=DR)
    qgT_f = gp.tile([128, 1], F32, name=f"qgT_f{br}")
    nc.scalar.activation(qgT_f[:], qg_ps[:], AF.Identity, bias=b2[:],
                         scale=1.0 / 64.0)
    sqg = gp.tile([128, 1], BF16, name=f"sqg{br}")
    nc.scalar.activation(sqg[:], qgT_f[:], AF.Square)
    ssg_ps = ps_m.tile([1, 1], F32, name=f"ssg{br}", tag="m")
    nc.tensor.matmul(ssg_ps[:], lhsT=tiles["onescb"][:], rhs=sqg[:],
                     start=True, stop=True)
    nrg = gp.tile([1, 1], F32, name=f"nrg{br}")
    nc.vector.tensor_scalar_max(nrg[:], ssg_ps[:], 1e-12)
    nrg2 = gp.tile([1, 1], F32, name=f"nrg2{br}")
    nc.scalar.activation(nrg2[:], nrg[:], AF.Ln)
    rng = gp.tile([1, 1], F32, name=f"rng{br}")
    nc.scalar.activation(rng[:], nrg2[:], AF.Exp, scale=-0.5)
    rngb_ps = ps_m.tile([128, 1], F32, name=f"rngb{br}", tag="m")
    nc.tensor.matmul(rngb_ps[:], lhsT=tiles["onesr"][:], rhs=rng[:],
                     start=True, stop=True)
    nc.vector.tensor_mul(dst_bf[:], qgT_f[:], rngb_ps[:])
    return dst_bf


def _body(nc, tc, env, cst):
    g = lambda k: env[k]
    tiles = {}

    # ---------------- inputs into SBUF ----------------
    tiles["xq8"] = cst.tile([128, CT * HW], F8, name="xq8")
    nc.sync.dma_start(tiles["xq8"][:], g("xq_d")[:])
    # scalar ring: only wd2 up front (needed at dt=1); the rest staggered
    tiles["wd2"] = cst.tile([128, D], F8, name="wd2")
    nc.scalar.dma_start(tiles["wd2"][:], g("wd2_d")[:])
    tiles["wg2"] = cst.tile([128, D], F8, name="wg2")
    tiles["wg1"] = cst.tile([128, CT * D], F8, name="wg1")
    # k-side tiles (DMAs staggered inside the q loop)
    tiles["xk8"] = cst.tile([128, CT * HW], F8, name="xk8")
    tiles["wd2m"] = cst.tile([128, D], F8, name="wd2m")
    tiles["wg1m"] = cst.tile([128, CT * D], F8, name="wg1m")
    tiles["wg2m"] = cst.tile([128, D], F8, name="wg2m")
    tiles["queueT8"] = cst.tile([128, QSH], F8, name="queueT8")
    # small consts on the gpsimd ring
    for nm, shp, dt in (("iota", (128, 1), F32), ("onesc", (128, 1), F32),
                        ("onesr", (1, 128), F32), ("ones8", (1, 8), BF16),
                        ("bd1", (128, DT), F32), ("bd1m", (128, DT), F32),
                        ("bd2", (128, 1), F32), ("bd2m", (128, 1), F32),
                        ("bg1r", (1, D), BF16), ("bg1mr", (1, D), BF16),
                        ("bg2", (128, 1), F32), ("bg2m", (128, 1), F32)):
        t = cst.tile(list(shp), dt, name=nm)
        nc.gpsimd.dma_start(t[:], g(nm + "_d")[:])
        tiles[nm] = t
    tiles["onescb"] = cst.tile([128, 1], BF16, name="onescb")
    nc.vector.tensor_copy(tiles["onescb"][:], tiles["onesc"][:])
    id_f = cst.tile([128, 128], F32, name="id_f")
    masks.make_identity(nc, id_f[:])
    id_b = cst.tile([128, 128], BF16, name="id_b")
    masks.make_identity(nc, id_b[:])

    # long-lived results
    for nm, shp, dt in (("qdT_bf", (128, HW), BF16),
                        ("kdT_bf", (128, HW), BF16),
                        ("qgT_bf", (128, 1), BF16),
                        ("kgT_bf", (128, 1), BF16),
                        ("qd8s", (128, 785), F8),
                        ("qall", (128, NT), F8),
                        ("qgall", (128, 8), F8),
                        ("matchT", (128, HW), BF16),
                        ("matchT8", (128, HW), F8),
                        ("zpart", (128, RT), F32),
                        ("fin", (1, 16), F32)):
        tiles[nm] = cst.tile(list(shp), dt, name=nm)
    nc.vector.memset(tiles["fin"][:], 0.0)

    pools = {}
    with tc.tile_pool(name="w1p", bufs=4) as pools["w1p"], \
         tc.tile_pool(name="w1k", bufs=4) as pools["w1k"], \
         tc.tile_pool(name="hp", bufs=2) as pools["hp"], \
         tc.tile_pool(name="l2s", bufs=2) as pools["l2s"], \
         tc.tile_pool(name="gp", bufs=1) as pools["gp"], \
         tc.tile_pool(name="pscr", bufs=2) as pools["pscr"], \
         tc.tile_pool(name="ps_h", bufs=2, space="PSUM") as pools["ps_h"], \
         tc.tile_pool(name="ps_m", bufs=2, space="PSUM") as pools["ps_m"]:

        # pooled feature sums for the q ghead, up front on the idle DVE
        # (the k-side pooling is emitted right after the xk8 DMA below)
        tiles["gsum0"] = cst.tile([128, CT], F32, name="gsum0")
        tiles["gsum1"] = cst.tile([128, CT], F32, name="gsum1")
        for ct in range(CT):
            pscr = pools["pscr"].tile([128, HW], F8, name="pscr")
            nc.vector.tensor_scalar(
                pscr[:], tiles["xq8"][:, ct * HW:(ct + 1) * HW],
                1.0, None, op0=ALU.mult, op1=ALU.add,
                accum_out=tiles["gsum0"][:, ct:ct + 1])
            if ct == 1:
                # gate the next wave of input DMAs behind this point of
                # the DVE stream: a dummy first-writer makes the (otherwise
                # dependency-free) loads wait, so they cannot steal HBM
                # bandwidth from the critical xq/wd1 stream at t=0
                for nm in ("xk8", "wg1", "wg2"):
                    nc.vector.memset(tiles[nm][:, 0:1], 0.0)

        # ========== q branch + its global head, then the AllGather ==========
        _dense_branch(nc, tc, env, cst, 0, pools, tiles)
        # prefetch the k-branch W1 pairs right behind the q pairs on the
        # sync ring (ring order keeps them off the critical q stream)
        w1k = []
        for dp in range(DT // 2):
            t = pools["w1k"].tile([128, 2 * C], F8, name="w1k")
            nc.sync.dma_start(t[:], g("wd1m_d")[dp, :, :])
            w1k.append(t)
        nc.vector.tensor_scalar_mul(tiles["qd8s"][:, 0:HW],
                                    tiles["qdT_bf"][:], 8.0)
        _ghead_branch(nc, tc, env, cst, 0, pools, tiles)
        nc.vector.tensor_scalar_mul(tiles["qd8s"][:, HW:HW + 1],
                                    tiles["qgT_bf"][:], 8.0)
        ag_in, ag_out = g("ag_in"), g("ag_out")
        nc.gpsimd.dma_start(ag_in[:].rearrange("(c p) -> c p", c=128),
                            tiles["qd8s"][:])
        nc.gpsimd.collective_compute(
            "AllGather", ALU.bypass, replica_groups=[list(range(N_CORES))],
            ins=[ag_in.opt()], outs=[ag_out.opt()])
        # low-urgency loads ride the gpsimd ring behind the AG staging,
        # so they cannot compete with the q/k-critical streams
        nc.gpsimd.dma_start(tiles["wg1m"][:], g("wg1m_d")[:])
        nc.gpsimd.dma_start(tiles["wg2m"][:], g("wg2m_d")[:])
        nc.gpsimd.dma_start(tiles["queueT8"][:], g("queueT_d")[:])

        # ========== k branch ==========
        _dense_branch(nc, tc, env, cst, 1, pools, tiles, w1tiles=w1k)

        # AG-output loads (wait on the collective, nothing else on sync)
        agv = ag_out[:].rearrange("(r c p) -> c r p", r=N_CORES, c=128)
        nc.sync.dma_start(
            tiles["qgall"][:].rearrange("c (r p) -> c r p", p=1),
            agv[:, :, HW:HW + 1])
        nc.sync.dma_start(
            tiles["qall"][:].rearrange("c (r p) -> c r p", r=N_CORES),
            agv[:, :, 0:HW])

        # ========== correspondence (own image, bf16) ==========
        qdT_bf, kdT_bf = tiles["qdT_bf"], tiles["kdT_bf"]
        with tc.tile_pool(name="cor", bufs=1) as cor, \
             tc.tile_pool(name="cor2", bufs=2) as cor2:
            sim_sb = cor.tile([128, 7 * HW], BF16, name="sim_sb")
            for i, (po, pn) in enumerate(PT):
                s_ps = pools["ps_h"].tile([128, HW], F32, name="s_ps",
                                          tag="h")
                for (o, n) in CH:
                    nc.tensor.matmul(s_ps[0:pn, o:o + n],
                                     lhsT=qdT_bf[:, po:po + pn],
                                     rhs=kdT_bf[:, o:o + n],
                                     start=True, stop=True)
                nc.scalar.activation(sim_sb[0:pn, i * HW:i * HW + HW],
                                     s_ps[0:pn, :], AF.Copy)
            mx8 = cor.tile([128, 8], F32, name="mx8")
            ix8 = cor.tile([128, 8], mybir.dt.uint32, name="ix8")
            ixf = cor.tile([128, 7], F32, name="ixf")
            for i, (po, pn) in enumerate(PT):
                nc.vector.max(mx8[0:pn, :], sim_sb[0:pn, i * HW:i * HW + HW])
                nc.vector.max_index(ix8[0:pn, :], mx8[0:pn, :],
                                    sim_sb[0:pn, i * HW:i * HW + HW])
                nc.vector.tensor_copy(ixf[0:pn, i:i + 1], ix8[0:pn, 0:1])

            # queue-negative matmuls here: the LDW-heavy PE work fills the
            # DVE-argmax window, keeping the PE busy (and the clock warm)
            qe_sb = tiles["qe_sb"] = cst.tile([128, 512], BF16, name="qe_sb")
            for grp in range(8):
                qe_ps = pools["ps_m"].tile([128, 64], F32, name="qe_ps",
                                           tag="m")
                for j in range(8):
                    qt = grp * 8 + j
                    nc.tensor.matmul(
                        qe_ps[:, j * 8:(j + 1) * 8],
                        lhsT=tiles["queueT8"][:, qt * 128:(qt + 1) * 128],
                        rhs=tiles["qgall"][:], start=(j == 0), stop=(j == 7))
                nc.scalar.activation(qe_sb[:, grp * 64:(grp + 1) * 64],
                                     qe_ps[:], AF.Exp, scale=ISC / 64.0)

            ir_sb = cor.tile([1, HW], F32, name="ir_sb")
            for i, (po, pn) in enumerate(PT):
                ir_ps = pools["ps_m"].tile([1, 128], F32, name="ir_ps",
                                           tag="m")
                nc.tensor.transpose(ir_ps[0:1, 0:pn], ixf[0:pn, i:i + 1],
                                    id_f[0:pn, 0:pn])
                nc.scalar.activation(ir_sb[0:1, po:po + pn],
                                     ir_ps[0:1, 0:pn], AF.Copy)
            ib_ps = pools["ps_m"].tile([128, HW], F32, name="ib_ps", tag="m")
            for (o, n) in CH:
                nc.tensor.matmul(ib_ps[:, o:o + n], lhsT=tiles["onesr"][:],
                                 rhs=ir_sb[:, o:o + n], start=True, stop=True)
            ib_sb = cor.tile([128, HW], F32, name="ib_sb")
            nc.scalar.activation(ib_sb[:], ib_ps[:], AF.Copy)
            # gather matched keys via one-hot matmuls; mt_ps stays resident
            # in ps_m while kt transposes rotate through ps_h
            mt_ps = pools["ps_m"].tile([128, HW], F32, name="mt_ps", tag="m")
            for i, (po, pn) in enumerate(PT):
                S = cor2.tile([128, HW], BF16, name="S")
                nc.vector.tensor_scalar(
                    S[0:pn, :], ib_sb[0:pn, :], tiles["iota"][0:pn, :],
                    float(po), op0=ALU.subtract, op1=ALU.is_equal)
                kt_ps = pools["ps_h"].tile([128, 128], BF16, name="kt_ps",
                                           tag="h")
                nc.tensor.transpose(kt_ps[0:pn, :], kdT_bf[:, po:po + pn],
                                    id_b[:, :])
                kt_sb = cor2.tile([128, 128], BF16, name="kt_sb")
                nc.scalar.activation(kt_sb[0:pn, :], kt_ps[0:pn, :], AF.Copy)
                for (o, n) in CH:
                    nc.tensor.matmul(mt_ps[:, o:o + n], lhsT=kt_sb[0:pn, :],
                                     rhs=S[0:pn, o:o + n],
                                     start=(i == 0), stop=(i == 6))
            nc.scalar.activation(tiles["matchT"][:], mt_ps[:], AF.Copy)
            nc.vector.tensor_scalar_mul(tiles["matchT8"][:], mt_ps[:], 8.0)

            # positives: diag = qd . matched (own rows), summed
            posm = cor.tile([128, HW], F32, name="posm")
            nc.vector.tensor_mul(posm[:], qdT_bf[:], tiles["matchT"][:])
            pos_ps = pools["ps_m"].tile([1, HW], F32, name="pos_ps", tag="m")
            for (o, n) in CH:
                nc.tensor.matmul(pos_ps[:, o:o + n], lhsT=tiles["onesc"][:],
                                 rhs=posm[:, o:o + n], start=True, stop=True)
            nc.vector.reduce_sum(tiles["fin"][0:1, 0:1], pos_ps[:],
                                 axis=mybir.AxisListType.X)

        # ========== gathered q: dense logits (ACT-bound tail) ==========
        with tc.tile_pool(name="escr", bufs=3) as escr:
            # dense logits, column shard: all 6272 q rows x own 784 keys;
            # per-row exp sums via DVE (keeps the ACT chain pure Exp).
            # The k global head (a latency chain of small PE/ACT/DVE hops)
            # is emitted early in the loop so it resolves under the exps.
            for t in range(RT):
                lg_ps = pools["ps_h"].tile([128, HW], F32, name="lg_ps",
                                           tag="h")
                for (o, n) in CH:
                    nc.tensor.matmul(
                        lg_ps[:, o:o + n],
                        lhsT=tiles["qall"][:, t * 128:(t + 1) * 128],
                        rhs=tiles["matchT8"][:, o:o + n],
                        start=True, stop=True)
                es = escr.tile([128, HW], BF16, name="es")
                nc.scalar.activation(es[:], lg_ps[:], AF.Exp,
                                     scale=ISC / 64.0)
                nc.vector.reduce_sum(tiles["zpart"][:, t:t + 1], es[:],
                                     axis=mybir.AxisListType.X)
                if t == 2:
                    _ghead_branch(nc, tc, env, cst, 1, pools, tiles)
                    lpm = pools["gp"].tile([128, 1], F32, name="lpm")
                    nc.vector.tensor_mul(lpm[:], tiles["qgT_bf"][:],
                                         tiles["kgT_bf"][:])
                    lp_ps = pools["ps_m"].tile([1, 1], F32, name="lp_ps",
                                               tag="m")
                    nc.tensor.matmul(lp_ps[:], lhsT=tiles["onesc"][:],
                                     rhs=lpm[:], start=True, stop=True)
                    nc.vector.tensor_copy(tiles["fin"][0:1, 1:2], lp_ps[:])
            qs_ps = pools["ps_m"].tile([1, 512], F32, name="qs_ps", tag="m")
            nc.tensor.matmul(qs_ps[:], lhsT=tiles["onescb"][:],
                             rhs=tiles["qe_sb"][:], start=True, stop=True)
            nc.vector.reduce_sum(tiles["fin"][0:1, 2:10],
                                 qs_ps[:].rearrange("p (t i) -> p i t", i=8),
                                 axis=mybir.AxisListType.X)

        nc.sync.dma_start(g("outz_d")[:], tiles["zpart"][:])
        nc.sync.dma_start(g("outs_d")[:], tiles["fin"][:])


def _prep_inputs(inputs):
    fq = np.asarray(inputs["feat_q"], np.float32).reshape(B, HW, C)
    fk = np.asarray(inputs["feat_k"], np.float32).reshape(B, HW, C)

    def xT8(x):  # (784, 1024) -> (128, 8*784) f8 with [c, ct*784+p]
        return np.ascontiguousarray(
            x.reshape(HW, CT, 128).transpose(2, 1, 0).reshape(128, CT * HW)
        ).astype(F8np)

    def w1tile(w):  # (1024, 2048) -> (8, 128, 2048) f8 x64, dt-pair major
        t = (w * 64.0).reshape(CT, 128, DT, 128).transpose(2, 1, 0, 3)
        t = t.reshape(DT // 2, 2, 128, C).transpose(0, 2, 1, 3)
        return np.ascontiguousarray(t.reshape(DT // 2, 128, 2 * C)
                                    ).astype(F8np)

    def w2tile(w):  # (2048, 128) -> (128, 2048) f8 x64
        return np.ascontiguousarray(
            (w * 64.0).reshape(DT, 128, 128).transpose(1, 0, 2)
            .reshape(128, D)).astype(F8np)

    def wg1tile(w):  # (1024, 2048) -> (128, 8*2048) f8 x64
        return np.ascontiguousarray(
            (w * 64.0).reshape(CT, 128, D).transpose(1, 0, 2)
            .reshape(128, CT * D)).astype(F8np)

    shared = {
        "wd1": w1tile(inputs["Wd1"]), "wd1m": w1tile(inputs["mWd1"]),
        "wd2": w2tile(inputs["Wd2"]), "wd2m": w2tile(inputs["mWd2"]),
        "wg1": wg1tile(inputs["Wg1"]), "wg1m": wg1tile(inputs["mWg1"]),
        "wg2": w2tile(inputs["Wg2"]), "wg2m": w2tile(inputs["mWg2"]),
        "bd1": np.ascontiguousarray(
            np.asarray(inputs["bd1"], np.float32).reshape(DT, 128).T),
        "bd1m": np.ascontiguousarray(
            np.asarray(inputs["mbd1"], np.float32).reshape(DT, 128).T),
        "bd2": np.asarray(inputs["bd2"], np.float32).reshape(128, 1),
        "bd2m": np.asarray(inputs["mbd2"], np.float32).reshape(128, 1),
        "bg1r": (np.asarray(inputs["bg1"], np.float32) * 4096.0
                 ).reshape(1, D).astype(BFnp),
        "bg1mr": (np.asarray(inputs["mbg1"], np.float32) * 4096.0
                  ).reshape(1, D).astype(BFnp),
        "bg2": np.asarray(inputs["bg2"], np.float32).reshape(128, 1),
        "bg2m": np.asarray(inputs["mbg2"], np.float32).reshape(128, 1),
        "iota": np.arange(128, dtype=np.float32).reshape(128, 1),
        "onesc": np.ones((128, 1), np.float32),
        "onesr": np.ones((1, 128), np.float32),
        "ones8": np.ones((1, 8), np.float32).astype(BFnp),
    }
    queue = np.asarray(inputs["queue"], np.float32)
    in_maps = []
    for c in range(N_CORES):
        m = dict(shared)
        m["xq"] = xT8(fq[c])
        m["xk"] = xT8(fk[c])
        m["queueT"] = np.ascontiguousarray(
            (queue[c * QSH:(c + 1) * QSH] * 8.0).T).astype(F8np)
        in_maps.append(m)
    return in_maps


_NC = None


def _get_nc():
    global _NC
    if _NC is None:
        _NC = _build()
    return _NC


def _host_combine(outz, outs):
    """outz: [8][128, 49] z-partials; outs: [8][1, 16] scalars.

    outs slots: [0] sum(qd.matched) over own rows, [1] own-image lpos,
    [2:10] partial sum(exp(l_neg/tau)) per image over the core's queue
    shard.  Dense z row r=t*128+p lives at outz[:, p, t].
    """
    outz = np.asarray(outz, np.float64)   # [8, 128, 49]
    outs = np.asarray(outs, np.float64)   # [8, 16]
    z = outz.sum(axis=0)                  # [128, 49]
    zrows = z.T.reshape(-1)               # row r = t*128+p
    pos_total = outs[:, 0].sum()
    l_d = (np.log(zrows).sum() - ISC * pos_total) / NT
    zq = outs[:, 2:10].sum(axis=0)        # [8]
    lpos = outs[np.arange(8), 1]          # core c owns image c
    lse = np.log(zq + np.exp(ISC * lpos))
    l_g = np.mean(lse - ISC * lpos)
    return np.float32((1.0 - LAM) * l_g + LAM * l_d).reshape(())


def kernel(**inputs) -> np.ndarray:
    nc = _get_nc()
    in_maps = _prep_inputs(inputs)
    res = bass_utils.run_bass_kernel_spmd(nc, in_maps,
                                          core_ids=list(range(N_CORES)))
    outz = np.stack([res.results[c]["outz"] for c in range(N_CORES)])
    outs = np.stack([res.results[c]["outs"].reshape(16)
                     for c in range(N_CORES)])
    return _host_combine(outz, outs)


# revision 60
# speedup vs baseline: 1.2727x; 1.2727x over previous
"""DenseCL loss kernel for 8 TRN2 NeuronCores (v2: fp8 DoubleRow + column-
sharded dense-InfoNCE logits).

Sharding: core c owns image c (dense head + correspondence + matched keys),
queue rows [c*8192, (c+1)*8192), and the COLUMN shard of the flat dense
logits: core c computes partial exp-sums over its own 784 matched-key
columns for ALL 6272 q rows; the host sums the per-core z partials.  The
only critical-path collective is a single early AllGather of the fp8
q_d (+ q_g) launched right after the q branch, hidden under the k branch.

Dense/global head matmuls run in fp8e4 with DoubleRow (2 contraction rows
per PE cell); weights are pre-scaled x64 on the host, the 1/64 folds into
the activation scale.  End-to-end fp8 rel-err vs the fp32 reference is
~5e-4 (validated in numpy), far under the 2e-2 gate.
"""
import sys

if "/opt/trn_rl_repo" not in sys.path:
    sys.path.insert(0, "/opt/trn_rl_repo")

import numpy as np
import ml_dtypes

import concourse.bass as bass
import concourse.bacc as bacc
import concourse.mybir as mybir
import concourse.tile as tile
from concourse import bass_utils, masks

F8np = ml_dtypes.float8_e4m3     # TRN FP8_EXP4-compatible (bias 7, max 240)
BFnp = ml_dtypes.bfloat16
F32 = mybir.dt.float32
BF16 = mybir.dt.bfloat16
F8 = mybir.dt.float8e4
DR = mybir.MatmulPerfMode.DoubleRow

N_CORES = 8
B, HW, C, D, P, Q = 8, 784, 1024, 2048, 128, 65536
QSH = Q // N_CORES          # 8192 queue rows per core
CT, DT = C // 128, D // 128  # 8, 16
NT = B * HW                 # 6272 total dense rows
RT = NT // 128              # 49 flat q-row tiles
TAU = 0.2
LAM = 0.5
ISC = 1.0 / TAU             # 5.0
AF = mybir.ActivationFunctionType
ALU = mybir.AluOpType

# 784 = 6*128 + 16 partition tiles (correspondence)
PT = [(i * 128, min(128, HW - i * 128)) for i in range(7)]
CH = [(0, 512), (512, HW - 512)]   # free-dim chunks of 784


def _patch_act_tables():
    """Force every activation we use onto the natural_log_exp_and_others
    table set so the kernel needs exactly one ACT_TABLE_LOAD."""
    import concourse.bacc as bacc_mod
    if getattr(bacc_mod, "_act_tables_patched", False):
        return
    from concourse import hw_specs
    orig = hw_specs.get_activation_tables
    ours = {AF.Exp, AF.Ln, AF.Relu, AF.Identity, AF.Copy, AF.Square}
    keep = "natural_log_exp_and_others"

    def patched(arch):
        tabs = orig(arch)
        assert keep in tabs and ours <= tabs[keep]
        return {name: (fns if name == keep else fns - ours)
                for name, fns in tabs.items()}

    bacc_mod.get_activation_tables = patched
    bacc_mod._act_tables_patched = True


def _build(do_compile=True):
    _patch_act_tables()
    nc = bacc.Bacc("TRN2", target_bir_lowering=False, debug=False,
                   num_devices=N_CORES)

    def inp(name, shape, dt):
        return nc.dram_tensor(name, list(shape), dt, kind="ExternalInput")

    env = {}
    env["xq_d"] = inp("xq", (128, CT * HW), F8)    # [c, ct*784+p]
    env["xk_d"] = inp("xk", (128, CT * HW), F8)
    env["wd1_d"] = inp("wd1", (DT // 2, 128, 2 * C), F8)  # dt-pairs, x64
    env["wd1m_d"] = inp("wd1m", (DT // 2, 128, 2 * C), F8)
    env["wd2_d"] = inp("wd2", (128, D), F8)        # [d, dt*128+p] x64
    env["wd2m_d"] = inp("wd2m", (128, D), F8)
    env["wg1_d"] = inp("wg1", (128, CT * D), F8)   # [c, ct*2048+d] x64
    env["wg1m_d"] = inp("wg1m", (128, CT * D), F8)
    env["wg2_d"] = inp("wg2", (128, D), F8)        # like wd2, x64
    env["wg2m_d"] = inp("wg2m", (128, D), F8)
    env["bd1_d"] = inp("bd1", (128, DT), F32)      # [r, dt] = bd1[dt*128+r]
    env["bd1m_d"] = inp("bd1m", (128, DT), F32)
    env["bd2_d"] = inp("bd2", (128, 1), F32)
    env["bd2m_d"] = inp("bd2m", (128, 1), F32)
    env["bg1r_d"] = inp("bg1r", (1, D), BF16)      # bg1 x4096 (bias row)
    env["bg1mr_d"] = inp("bg1mr", (1, D), BF16)
    env["bg2_d"] = inp("bg2", (128, 1), F32)
    env["bg2m_d"] = inp("bg2m", (128, 1), F32)
    env["queueT_d"] = inp("queueT", (128, QSH), F8)  # 8*queue[c0+j, ch]
    env["iota_d"] = inp("iota", (128, 1), F32)
    env["onesc_d"] = inp("onesc", (128, 1), F32)
    env["onesr_d"] = inp("onesr", (1, 128), F32)
    env["ones8_d"] = inp("ones8", (1, 8), BF16)

    env["outz_d"] = nc.dram_tensor("outz", [128, RT], F32,
                                   kind="ExternalOutput")
    env["outs_d"] = nc.dram_tensor("outs", [1, 24], F32,
                                   kind="ExternalOutput")

    with tile.TileContext(nc) as tc:
        with tc.tile_pool(name="dramp", bufs=1, space="DRAM") as dpool:
            env["ag_in"] = dpool.tile([128 * 800], F8, name="ag_in")
            env["ag_out"] = dpool.tile([N_CORES * 128 * 800], F8,
                                       name="ag_out", addr_space="Shared")
            with tc.tile_pool(name="cst", bufs=1) as cst:
                _body(nc, tc, env, cst)
    if do_compile:
        nc.compile()
    return nc


def _dense_branch(nc, tc, env, cst, br, pools, tiles, w1tiles=None):
    """One dense-head branch (q: br=0, k: br=1) -> normalized [128, HW]."""
    g = lambda k: env[k]
    sfx = "" if br == 0 else "m"
    x_sb = tiles["xq8" if br == 0 else "xk8"]
    w1_d = g("wd1" + sfx + "_d")
    w2_sb = tiles["wd2" + sfx]
    b1 = tiles["bd1" + sfx]
    b2 = tiles["bd2" + sfx]
    dst_bf = tiles["qdT_bf" if br == 0 else "kdT_bf"]
    w1p, hp, l2s, ps_h, ps_m = (pools["w1p"], pools["hp"], pools["l2s"],
                                pools["ps_h"], pools["ps_m"])

    xv = x_sb[:].rearrange("c (t p) -> c t p", t=CT)
    qd_ps = ps_m.tile([128, HW], F32, name=f"qd_ps{br}", tag="m")
    hq = None
    hq_prev = None

    def l2_pair(hsrc, dp):
        w2v = w2_sb[:].rearrange("c (t d) -> c t d", t=DT)
        hv = hsrc[:].rearrange("c (j p) -> c j p", j=2)
        for (o, n) in CH:
            nc.tensor.matmul(
                qd_ps[:, o:o + n],
                lhsT=w2v[:, 2 * dp:2 * dp + 2, :],
                rhs=hv[:, :, o:o + n],
                start=(dp == 0), stop=(dp == DT // 2 - 1),
                perf_mode=DR)

    for dt in range(DT):
        # one DMA per dt-pair: 2 KB/partition transfers use the DMA
        # engines far better than 1 KB ones
        if dt % 2 == 0:
            w1t = w1tiles[dt // 2] if w1tiles else w1p.tile(
                [128, 2 * C], F8, name=f"w1t{br}")
            if not w1tiles:
                nc.sync.dma_start(w1t[:], w1_d[dt // 2, :, :])
        h_ps = ps_h.tile([128, HW], F32, name="h_ps", tag="h")
        wv = w1t[:, (dt % 2) * C:(dt % 2 + 1) * C].rearrange(
            "c (t d) -> c t d", t=CT)
        for kp in range(CT // 2):
            for (o, n) in CH:
                nc.tensor.matmul(
                    h_ps[:, o:o + n],
                    lhsT=wv[:, 2 * kp:2 * kp + 2, :],
                    rhs=xv[:, 2 * kp:2 * kp + 2, o:o + n],
                    start=(kp == 0), stop=(kp == CT // 2 - 1),
                    perf_mode=DR)
        # L2 for the pair two dts back: by now its relus have finished, so
        # the PE never stalls waiting on the ACT chain
        if dt % 2 == 0 and dt >= 2:
            l2_pair(hq, dt // 2 - 1)
        if dt % 2 == 0:
            hq = hp.tile([128, 2 * HW], F8, name=f"hq{br}")
        nc.scalar.activation(hq[:, (dt % 2) * HW:(dt % 2 + 1) * HW],
                             h_ps[:], AF.Relu, bias=b1[:, dt:dt + 1],
                             scale=1.0 / 64.0)
        if br == 0:
            # stagger the k-branch / tail input DMAs on the scalar ring so
            # they don't compete with the q-critical loads
            if dt == 2:
                nc.gpsimd.dma_start(tiles["xk8"][:], g("xk_d")[:])
                # pooled sums for the k ghead: must be emitted after the
                # xk8 DMA (program order defines the dependency), runs on
                # the otherwise idle DVE during the q branch
                gsum = tiles["gsum1"]
                for ct in range(CT):
                    pscr = pools["pscr"].tile([128, HW], F8, name="pscr")
                    nc.vector.tensor_scalar(
                        pscr[:], tiles["xk8"][:, ct * HW:(ct + 1) * HW],
                        1.0, None, op0=ALU.mult, op1=ALU.add,
                        accum_out=gsum[:, ct:ct + 1])
                nc.vector.tensor_scalar_mul(
                    tiles["qd8s"][:, HW + 8:HW + 16], gsum[:], 64.0 / HW)

            elif dt == 4:
                nc.scalar.dma_start(tiles["wg1"][:], g("wg1_d")[:])
                nc.scalar.dma_start(tiles["wg2"][:], g("wg2_d")[:])
            elif dt == 8:
                nc.scalar.dma_start(tiles["wd2m"][:], g("wd2m_d")[:])

    l2_pair(hq, DT // 2 - 1)

    # bias + l2 normalize along channels (partition dim)
    qdT_f = l2s.tile([128, HW], F32, name=f"qdT_f{br}")
    nc.scalar.activation(qdT_f[:], qd_ps[:], AF.Identity, bias=b2[:],
                         scale=1.0 / 64.0)
    sq = l2s.tile([128, HW], BF16, name=f"sq{br}")
    nc.scalar.activation(sq[:], qdT_f[:], AF.Square)
    ssq_ps = ps_m.tile([1, HW], F32, name=f"ssq{br}", tag="m")
    for (o, n) in CH:
        nc.tensor.matmul(ssq_ps[:, o:o + n], lhsT=tiles["onescb"][:],
                         rhs=sq[:, o:o + n], start=True, stop=True)
    nrm = l2s.tile([1, HW], F32, name=f"nrm{br}")
    nc.vector.tensor_scalar_max(nrm[:], ssq_ps[:], 1e-12)
    nrm2 = l2s.tile([1, HW], F32, name=f"nrm2{br}")
    nc.scalar.activation(nrm2[:], nrm[:], AF.Ln)
    rn = l2s.tile([1, HW], F32, name=f"rn{br}")
    nc.scalar.activation(rn[:], nrm2[:], AF.Exp, scale=-0.5)
    rnb_ps = ps_m.tile([128, HW], F32, name=f"rnb{br}", tag="m")
    for (o, n) in CH:
        nc.tensor.matmul(rnb_ps[:, o:o + n], lhsT=tiles["onesr"][:],
                         rhs=rn[:, o:o + n], start=True, stop=True)
    nc.vector.tensor_mul(dst_bf[:], qdT_f[:], rnb_ps[:])
    return dst_bf


def _ghead_all(nc, tc, env, cst, br, pools, tiles):
    """Global head for ALL 8 images (q: br=0, k: br=1) from the gathered
    pooled features (g*64, fp8, staged at stride 16 with img innermost)."""
    g = lambda k: env[k]
    sfx = "" if br == 0 else "m"
    w1_sb = tiles["wg1" + sfx]
    w2_sb = tiles["wg2" + sfx]
    b1r = tiles["bg1r" if br == 0 else "bg1mr"]
    b2 = tiles["bg2" + sfx]
    gp, ps_m = pools["gp"], pools["ps_m"]
    ones8 = tiles["ones8"]

    gq_v = tiles[f"gq_all{br}"][:].rearrange("c (t s) -> c t s", s=16)
    # L1: h_g[8, 2048] = (g*64) @ (Wg1*64) / 4096 + bg1, in 512-chunks
    hgb = gp.tile([8, D], BF16, name=f"hgb{br}")
    w1v = w1_sb[:].rearrange("c (t d) -> c t d", t=CT)
    for ch in range(4):
        hg_ps = ps_m.tile([8, 512], F32, name=f"hg{br}", tag="m")
        for kp in range(CT // 2):
            nc.tensor.matmul(
                hg_ps[:], lhsT=gq_v[:, 2 * kp:2 * kp + 2, 0:8],
                rhs=w1v[:, 2 * kp:2 * kp + 2, ch * 512:(ch + 1) * 512],
                start=(kp == 0), stop=False, perf_mode=DR)
        nc.tensor.matmul(hg_ps[:], lhsT=ones8[0:1, 0:8],
                         rhs=b1r[0:1, ch * 512:(ch + 1) * 512],
                         start=False, stop=True)
        nc.scalar.activation(hgb[0:8, ch * 512:(ch + 1) * 512], hg_ps[:],
                             AF.Relu, scale=1.0 / 4096.0)
    # transpose h_g -> [128, dt*8+img] via small K=8 matmuls with identity
    hgt_ps = ps_m.tile([128, DT * 8], F32, name=f"hgt{br}", tag="m")
    for dt in range(DT):
        nc.tensor.matmul(hgt_ps[:, dt * 8:(dt + 1) * 8],
                         lhsT=hgb[0:8, dt * 128:(dt + 1) * 128],
                         rhs=tiles["id_b"][0:8, 0:8],
                         start=(dt == 0), stop=(dt == DT - 1))
    hgt8 = gp.tile([128, DT * 16], F8, name=f"hgt8{br}")
    hgt_v = hgt8[:].rearrange("c (t s) -> c t s", s=16)
    nc.scalar.activation(hgt_v[:, :, 0:8],
                         hgt_ps[:].rearrange("c (t i) -> c t i", i=8),
                         AF.Copy)
    # L2: q_g[128, 8]
    qg_ps = ps_m.tile([128, 8], F32, name=f"qg{br}", tag="m")
    w2v = w2_sb[:].rearrange("c (t d) -> c t d", t=DT)
    for dp in range(DT // 2):
        nc.tensor.matmul(qg_ps[:], lhsT=w2v[:, 2 * dp:2 * dp + 2, :],
                         rhs=hgt_v[:, 2 * dp:2 * dp + 2, 0:8],
                         start=(dp == 0), stop=(dp == DT // 2 - 1),
                         perf_mode=DR)
    qgT_f = gp.tile([128, 8], F32, name=f"qgT_f{br}")
    nc.scalar.activation(qgT_f[:], qg_ps[:], AF.Identity, bias=b2[:],
                         scale=1.0 / 64.0)
    sqg = gp.tile([128, 8], BF16, name=f"sqg{br}")
    nc.scalar.activation(sqg[:], qgT_f[:], AF.Square)
    ssg_ps = ps_m.tile([1, 8], F32, name=f"ssg{br}", tag="m")
    nc.tensor.matmul(ssg_ps[:], lhsT=tiles["onescb"][:], rhs=sqg[:],
                     start=True, stop=True)
    nrg = gp.tile([1, 8], F32, name=f"nrg{br}")
    nc.vector.tensor_scalar_max(nrg[:], ssg_ps[:], 1e-12)
    nrg2 = gp.tile([1, 8], F32, name=f"nrg2{br}")
    nc.scalar.activation(nrg2[:], nrg[:], AF.Ln)
    rng = gp.tile([1, 8], F32, name=f"rng{br}")
    nc.scalar.activation(rng[:], nrg2[:], AF.Exp, scale=-0.5)
    rngb_ps = ps_m.tile([128, 8], F32, name=f"rngb{br}", tag="m")
    nc.tensor.matmul(rngb_ps[:], lhsT=tiles["onesr"][:], rhs=rng[:],
                     start=True, stop=True)
    dst_bf = tiles["qgT_bf" if br == 0 else "kgT_bf"]
    nc.vector.tensor_mul(dst_bf[:], qgT_f[:], rngb_ps[:])
    return dst_bf


def _body(nc, tc, env, cst):
    g = lambda k: env[k]
    tiles = {}

    # ---------------- inputs into SBUF ----------------
    tiles["xq8"] = cst.tile([128, CT * HW], F8, name="xq8")
    nc.sync.dma_start(tiles["xq8"][:], g("xq_d")[:])
    # scalar ring: only wd2 up front (needed at dt=1); the rest staggered
    tiles["wd2"] = cst.tile([128, D], F8, name="wd2")
    nc.scalar.dma_start(tiles["wd2"][:], g("wd2_d")[:])
    tiles["wg2"] = cst.tile([128, D], F8, name="wg2")
    tiles["wg1"] = cst.tile([128, CT * D], F8, name="wg1")
    # k-side tiles (DMAs staggered inside the q loop)
    tiles["xk8"] = cst.tile([128, CT * HW], F8, name="xk8")
    tiles["wd2m"] = cst.tile([128, D], F8, name="wd2m")
    tiles["wg1m"] = cst.tile([128, CT * D], F8, name="wg1m")
    tiles["wg2m"] = cst.tile([128, D], F8, name="wg2m")
    tiles["queueT8"] = cst.tile([128, QSH], F8, name="queueT8")
    # small consts on the gpsimd ring
    for nm, shp, dt in (("iota", (128, 1), F32), ("onesc", (128, 1), F32),
                        ("onesr", (1, 128), F32), ("ones8", (1, 8), BF16),
                        ("bd1", (128, DT), F32), ("bd1m", (128, DT), F32),
                        ("bd2", (128, 1), F32), ("bd2m", (128, 1), F32),
                        ("bg1r", (1, D), BF16), ("bg1mr", (1, D), BF16),
                        ("bg2", (128, 1), F32), ("bg2m", (128, 1), F32)):
        t = cst.tile(list(shp), dt, name=nm)
        nc.gpsimd.dma_start(t[:], g(nm + "_d")[:])
        tiles[nm] = t
    tiles["onescb"] = cst.tile([128, 1], BF16, name="onescb")
    nc.vector.tensor_copy(tiles["onescb"][:], tiles["onesc"][:])
    tiles["onesB"] = cst.tile([128, 128], F32, name="onesB")
    nc.vector.memset(tiles["onesB"][:], 1.0)
    id_f = cst.tile([128, 128], F32, name="id_f")
    masks.make_identity(nc, id_f[:])
    id_b = cst.tile([128, 128], BF16, name="id_b")
    masks.make_identity(nc, id_b[:])

    # long-lived results
    for nm, shp, dt in (("qdT_bf", (128, HW), BF16),
                        ("kdT_bf", (128, HW), BF16),
                        ("qgT_bf", (128, 8), BF16),
                        ("kgT_bf", (128, 8), BF16),
                        ("qd8s", (128, 800), F8),
                        ("qall", (128, NT), F8),
                        ("qg8all", (128, 8), F8),
                        ("gq_all0", (128, CT * 16), F8),
                        ("gq_all1", (128, CT * 16), F8),
                        ("matchT", (128, HW), BF16),
                        ("matchT8", (128, HW), F8),
                        ("zpart", (128, RT), F32),
                        ("fin", (1, 24), F32)):
        tiles[nm] = cst.tile(list(shp), dt, name=nm)
    tiles["id_b"] = id_b
    nc.vector.memset(tiles["fin"][:], 0.0)

    pools = {}
    with tc.tile_pool(name="w1p", bufs=4) as pools["w1p"], \
         tc.tile_pool(name="w1k", bufs=4) as pools["w1k"], \
         tc.tile_pool(name="hp", bufs=2) as pools["hp"], \
         tc.tile_pool(name="l2s", bufs=2) as pools["l2s"], \
         tc.tile_pool(name="gp", bufs=1) as pools["gp"], \
         tc.tile_pool(name="pscr", bufs=2) as pools["pscr"], \
         tc.tile_pool(name="ps_h", bufs=2, space="PSUM") as pools["ps_h"], \
         tc.tile_pool(name="ps_m", bufs=2, space="PSUM") as pools["ps_m"]:

        # pooled feature sums for the q ghead, up front on the idle DVE
        # (the k-side pooling is emitted right after the xk8 DMA below)
        tiles["gsum0"] = cst.tile([128, CT], F32, name="gsum0")
        tiles["gsum1"] = cst.tile([128, CT], F32, name="gsum1")
        for ct in range(CT):
            pscr = pools["pscr"].tile([128, HW], F8, name="pscr")
            nc.vector.tensor_scalar(
                pscr[:], tiles["xq8"][:, ct * HW:(ct + 1) * HW],
                1.0, None, op0=ALU.mult, op1=ALU.add,
                accum_out=tiles["gsum0"][:, ct:ct + 1])
            if ct == 1:
                # gate the next wave of input DMAs behind this point of
                # the DVE stream: a dummy first-writer makes the (otherwise
                # dependency-free) loads wait, so they cannot steal HBM
                # bandwidth from the critical xq/wd1 stream at t=0
                for nm in ("xk8", "wg1", "wg2"):
                    nc.vector.memset(tiles[nm][:, 0:1], 0.0)
        # own pooled features (g*64, f8) ride along in the AllGather
        nc.vector.tensor_scalar_mul(tiles["qd8s"][:, HW:HW + 8],
                                    tiles["gsum0"][:], 64.0 / HW)

        # ========== q branch, then the AllGather ==========
        _dense_branch(nc, tc, env, cst, 0, pools, tiles)
        # prefetch the k-branch W1 pairs right behind the q pairs on the
        # sync ring (ring order keeps them off the critical q stream)
        w1k = []
        for dp in range(DT // 2):
            t = pools["w1k"].tile([128, 2 * C], F8, name="w1k")
            nc.sync.dma_start(t[:], g("wd1m_d")[dp, :, :])
            w1k.append(t)
        nc.vector.tensor_scalar_mul(tiles["qd8s"][:, 0:HW],
                                    tiles["qdT_bf"][:], 8.0)
        ag_in, ag_out = g("ag_in"), g("ag_out")
        nc.gpsimd.dma_start(ag_in[:].rearrange("(c p) -> c p", c=128),
                            tiles["qd8s"][:])
        nc.gpsimd.collective_compute(
            "AllGather", ALU.bypass, replica_groups=[list(range(N_CORES))],
            ins=[ag_in.opt()], outs=[ag_out.opt()])
        # low-urgency loads ride the gpsimd ring behind the AG staging,
        # so they cannot compete with the q/k-critical streams
        nc.gpsimd.dma_start(tiles["wg1m"][:], g("wg1m_d")[:])
        nc.gpsimd.dma_start(tiles["wg2m"][:], g("wg2m_d")[:])
        nc.gpsimd.dma_start(tiles["queueT8"][:], g("queueT_d")[:])

        # ========== k branch ==========
        _dense_branch(nc, tc, env, cst, 1, pools, tiles, w1tiles=w1k)

        # AG-output loads (wait on the collective, nothing else on sync).
        # Pooled features land in stride-16 staging (img innermost); the
        # big qall block is split per rank so the first logits tiles can
        # start as soon as the head of the buffer lands.
        agv = ag_out[:].rearrange("(r c p) -> c r p", r=N_CORES, c=128)
        for br2, off in ((0, HW), (1, HW + 8)):
            for t in range(CT):
                nc.sync.dma_start(
                    tiles[f"gq_all{br2}"][:, t * 16:t * 16 + 8],
                    agv[:, :, off + t])
        for r in range(N_CORES):
            nc.sync.dma_start(
                tiles["qall"][:, r * HW:(r + 1) * HW], agv[:, r, 0:HW])

        # ========== correspondence (own image, bf16) ==========
        qdT_bf, kdT_bf = tiles["qdT_bf"], tiles["kdT_bf"]
        with tc.tile_pool(name="cor", bufs=1) as cor, \
             tc.tile_pool(name="cor2", bufs=2) as cor2:
            sim_sb = cor.tile([128, 7 * HW], BF16, name="sim_sb")
            for i, (po, pn) in enumerate(PT):
                s_ps = pools["ps_h"].tile([128, HW], F32, name="s_ps",
                                          tag="h")
                for (o, n) in CH:
                    nc.tensor.matmul(s_ps[0:pn, o:o + n],
                                     lhsT=qdT_bf[:, po:po + pn],
                                     rhs=kdT_bf[:, o:o + n],
                                     start=True, stop=True)
                nc.scalar.activation(sim_sb[0:pn, i * HW:i * HW + HW],
                                     s_ps[0:pn, :], AF.Copy)
            mx8 = cor.tile([128, 8], F32, name="mx8")
            ix8 = cor.tile([128, 8], mybir.dt.uint32, name="ix8")
            ixf = cor.tile([128, 7], F32, name="ixf")
            for i, (po, pn) in enumerate(PT):
                nc.vector.max(mx8[0:pn, :], sim_sb[0:pn, i * HW:i * HW + HW])
                nc.vector.max_index(ix8[0:pn, :], mx8[0:pn, :],
                                    sim_sb[0:pn, i * HW:i * HW + HW])
                nc.vector.tensor_copy(ixf[0:pn, i:i + 1], ix8[0:pn, 0:1])

            # global heads for all 8 images (from the gathered pools) plus
            # the queue-negative matmuls: this PE work fills the DVE-argmax
            # window, and only needs the AllGather, which landed by now
            _ghead_all(nc, tc, env, cst, 0, pools, tiles)
            _ghead_all(nc, tc, env, cst, 1, pools, tiles)
            nc.vector.tensor_scalar_mul(tiles["qg8all"][:],
                                        tiles["qgT_bf"][:], 8.0)
            lpm = pools["gp"].tile([128, 8], F32, name="lpm")
            nc.vector.tensor_mul(lpm[:], tiles["qgT_bf"][:],
                                 tiles["kgT_bf"][:])
            lp_ps = pools["ps_m"].tile([1, 8], F32, name="lp_ps", tag="m")
            nc.tensor.matmul(lp_ps[:], lhsT=tiles["onesc"][:], rhs=lpm[:],
                             start=True, stop=True)
            nc.vector.tensor_copy(tiles["fin"][0:1, 1:9], lp_ps[:])
            qe_sb = tiles["qe_sb"] = cst.tile([128, 512], BF16, name="qe_sb")
            for grp in range(8):
                qe_ps = pools["ps_m"].tile([128, 64], F32, name="qe_ps",
                                           tag="m")
                for j in range(8):
                    qt = grp * 8 + j
                    nc.tensor.matmul(
                        qe_ps[:, j * 8:(j + 1) * 8],
                        lhsT=tiles["queueT8"][:, qt * 128:(qt + 1) * 128],
                        rhs=tiles["qg8all"][:], start=(j == 0), stop=(j == 7))
                nc.scalar.activation(qe_sb[:, grp * 64:(grp + 1) * 64],
                                     qe_ps[:], AF.Exp, scale=ISC / 64.0)

            # broadcast the per-pixel argmax indices down the partitions
            ir_sb = cor.tile([1, HW], F32, name="ir_sb")
            for i, (po, pn) in enumerate(PT):
                ir_ps = pools["ps_m"].tile([1, 128], F32, name="ir_ps",
                                           tag="m")
                nc.tensor.transpose(ir_ps[0:1, 0:pn], ixf[0:pn, i:i + 1],
                                    id_f[0:pn, 0:pn])
                nc.scalar.activation(ir_sb[0:1, po:po + pn],
                                     ir_ps[0:1, 0:pn], AF.Copy)
            ib_ps = pools["ps_m"].tile([128, HW], F32, name="ib_ps", tag="m")
            for (o, n) in CH:
                nc.tensor.matmul(ib_ps[:, o:o + n], lhsT=tiles["onesr"][:],
                                 rhs=ir_sb[:, o:o + n], start=True, stop=True)
            ib_sb = cor.tile([128, HW], F32, name="ib_sb")
            nc.scalar.activation(ib_sb[:], ib_ps[:], AF.Copy)
            # gather matched keys via one-hot matmuls; mt_ps stays resident
            # in ps_m while kt transposes rotate through ps_h
            mt_ps = pools["ps_m"].tile([128, HW], F32, name="mt_ps", tag="m")
            for i, (po, pn) in enumerate(PT):
                S = cor2.tile([128, HW], BF16, name="S")
                nc.vector.tensor_scalar(
                    S[0:pn, :], ib_sb[0:pn, :], tiles["iota"][0:pn, :],
                    float(po), op0=ALU.subtract, op1=ALU.is_equal)
                kt_ps = pools["ps_h"].tile([128, 128], BF16, name="kt_ps",
                                           tag="h")
                nc.tensor.transpose(kt_ps[0:pn, :], kdT_bf[:, po:po + pn],
                                    id_b[:, :])
                kt_sb = cor2.tile([128, 128], BF16, name="kt_sb")
                nc.scalar.activation(kt_sb[0:pn, :], kt_ps[0:pn, :], AF.Copy)
                for (o, n) in CH:
                    nc.tensor.matmul(mt_ps[:, o:o + n], lhsT=kt_sb[0:pn, :],
                                     rhs=S[0:pn, o:o + n],
                                     start=(i == 0), stop=(i == 6))
            nc.scalar.activation(tiles["matchT"][:], mt_ps[:], AF.Copy)
            nc.vector.tensor_scalar_mul(tiles["matchT8"][:], mt_ps[:], 8.0)

            # positives: diag = qd . matched (own rows), summed
            posm = cor.tile([128, HW], F32, name="posm")
            nc.vector.tensor_mul(posm[:], qdT_bf[:], tiles["matchT"][:])
            pos_ps = pools["ps_m"].tile([1, HW], F32, name="pos_ps", tag="m")
            for (o, n) in CH:
                nc.tensor.matmul(pos_ps[:, o:o + n], lhsT=tiles["onesc"][:],
                                 rhs=posm[:, o:o + n], start=True, stop=True)
            nc.vector.reduce_sum(tiles["fin"][0:1, 0:1], pos_ps[:],
                                 axis=mybir.AxisListType.X)

        # ========== gathered q: dense logits (ACT-bound tail) ==========
        with tc.tile_pool(name="escr", bufs=3) as escr:
            # dense logits, column shard: all 6272 q rows x own 784 keys;
            # per-row exp sums via DVE (keeps the ACT chain pure Exp).
            # The k global head (a latency chain of small PE/ACT/DVE hops)
            # is emitted early in the loop so it resolves under the exps.
            for t in range(RT):
                lg_ps = pools["ps_h"].tile([128, HW], F32, name="lg_ps",
                                           tag="h")
                for (o, n) in CH:
                    nc.tensor.matmul(
                        lg_ps[:, o:o + n],
                        lhsT=tiles["qall"][:, t * 128:(t + 1) * 128],
                        rhs=tiles["matchT8"][:, o:o + n],
                        start=True, stop=True)
                es = escr.tile([128, HW], BF16, name="es")
                nc.scalar.activation(es[:], lg_ps[:], AF.Exp,
                                     scale=ISC / 64.0)
                nc.vector.reduce_sum(tiles["zpart"][:, t:t + 1], es[:],
                                     axis=mybir.AxisListType.X)
            qs_ps = pools["ps_m"].tile([1, 512], F32, name="qs_ps", tag="m")
            nc.tensor.matmul(qs_ps[:], lhsT=tiles["onescb"][:],
                             rhs=tiles["qe_sb"][:], start=True, stop=True)
            nc.vector.reduce_sum(tiles["fin"][0:1, 9:17],
                                 qs_ps[:].rearrange("p (t i) -> p i t", i=8),
                                 axis=mybir.AxisListType.X)

        nc.sync.dma_start(g("outz_d")[:], tiles["zpart"][:])
        nc.sync.dma_start(g("outs_d")[:], tiles["fin"][:])


def _prep_inputs(inputs):
    fq = np.asarray(inputs["feat_q"], np.float32).reshape(B, HW, C)
    fk = np.asarray(inputs["feat_k"], np.float32).reshape(B, HW, C)

    def xT8(x):  # (784, 1024) -> (128, 8*784) f8 with [c, ct*784+p]
        return np.ascontiguousarray(
            x.reshape(HW, CT, 128).transpose(2, 1, 0).reshape(128, CT * HW)
        ).astype(F8np)

    def w1tile(w):  # (1024, 2048) -> (8, 128, 2048) f8 x64, dt-pair major
        t = (w * 64.0).reshape(CT, 128, DT, 128).transpose(2, 1, 0, 3)
        t = t.reshape(DT // 2, 2, 128, C).transpose(0, 2, 1, 3)
        return np.ascontiguousarray(t.reshape(DT // 2, 128, 2 * C)
                                    ).astype(F8np)

    def w2tile(w):  # (2048, 128) -> (128, 2048) f8 x64
        return np.ascontiguousarray(
            (w * 64.0).reshape(DT, 128, 128).transpose(1, 0, 2)
            .reshape(128, D)).astype(F8np)

    def wg1tile(w):  # (1024, 2048) -> (128, 8*2048) f8 x64
        return np.ascontiguousarray(
            (w * 64.0).reshape(CT, 128, D).transpose(1, 0, 2)
            .reshape(128, CT * D)).astype(F8np)

    shared = {
        "wd1": w1tile(inputs["Wd1"]), "wd1m": w1tile(inputs["mWd1"]),
        "wd2": w2tile(inputs["Wd2"]), "wd2m": w2tile(inputs["mWd2"]),
        "wg1": wg1tile(inputs["Wg1"]), "wg1m": wg1tile(inputs["mWg1"]),
        "wg2": w2tile(inputs["Wg2"]), "wg2m": w2tile(inputs["mWg2"]),
        "bd1": np.ascontiguousarray(
            np.asarray(inputs["bd1"], np.float32).reshape(DT, 128).T),
        "bd1m": np.ascontiguousarray(
            np.asarray(inputs["mbd1"], np.float32).reshape(DT, 128).T),
        "bd2": np.asarray(inputs["bd2"], np.float32).reshape(128, 1),
        "bd2m": np.asarray(inputs["mbd2"], np.float32).reshape(128, 1),
        "bg1r": (np.asarray(inputs["bg1"], np.float32) * 4096.0
                 ).reshape(1, D).astype(BFnp),
        "bg1mr": (np.asarray(inputs["mbg1"], np.float32) * 4096.0
                  ).reshape(1, D).astype(BFnp),
        "bg2": np.asarray(inputs["bg2"], np.float32).reshape(128, 1),
        "bg2m": np.asarray(inputs["mbg2"], np.float32).reshape(128, 1),
        "iota": np.arange(128, dtype=np.float32).reshape(128, 1),
        "onesc": np.ones((128, 1), np.float32),
        "onesr": np.ones((1, 128), np.float32),
        "ones8": np.ones((1, 8), np.float32).astype(BFnp),
    }
    queue = np.asarray(inputs["queue"], np.float32)
    in_maps = []
    for c in range(N_CORES):
        m = dict(shared)
        m["xq"] = xT8(fq[c])
        m["xk"] = xT8(fk[c])
        m["queueT"] = np.ascontiguousarray(
            (queue[c * QSH:(c + 1) * QSH] * 8.0).T).astype(F8np)
        in_maps.append(m)
    return in_maps


_NC = None


def _get_nc():
    global _NC
    if _NC is None:
        _NC = _build()
    return _NC


def _host_combine(outz, outs):
    """outz: [8][128, 49] z-partials; outs: [8][1, 24] scalars.

    outs slots: [0] sum(qd.matched) over own rows, [1:9] lpos per image
    (replicated on every core), [9:17] partial sum(exp(l_neg/tau)) per
    image over the core's queue shard.  Dense z row r=t*128+p lives at
    outz[:, p, t].
    """
    outz = np.asarray(outz, np.float64)   # [8, 128, 49]
    outs = np.asarray(outs, np.float64)   # [8, 24]
    z = outz.sum(axis=0)                  # [128, 49]
    zrows = z.T.reshape(-1)               # row r = t*128+p
    pos_total = outs[:, 0].sum()
    l_d = (np.log(zrows).sum() - ISC * pos_total) / NT
    zq = outs[:, 9:17].sum(axis=0)        # [8]
    lpos = outs[0, 1:9]                   # replicated
    lse = np.log(zq + np.exp(ISC * lpos))
    l_g = np.mean(lse - ISC * lpos)
    return np.float32((1.0 - LAM) * l_g + LAM * l_d).reshape(())


def kernel(**inputs) -> np.ndarray:
    nc = _get_nc()
    in_maps = _prep_inputs(inputs)
    res = bass_utils.run_bass_kernel_spmd(nc, in_maps,
                                          core_ids=list(range(N_CORES)))
    outz = np.stack([res.results[c]["outz"] for c in range(N_CORES)])
    outs = np.stack([res.results[c]["outs"].reshape(24)
                     for c in range(N_CORES)])
    return _host_combine(outz, outs)


# revision 64
# speedup vs baseline: 1.3441x; 1.0561x over previous
"""DenseCL loss kernel for 8 TRN2 NeuronCores (v2: fp8 DoubleRow + column-
sharded dense-InfoNCE logits).

Sharding: core c owns image c (dense head + correspondence + matched keys),
queue rows [c*8192, (c+1)*8192), and the COLUMN shard of the flat dense
logits: core c computes partial exp-sums over its own 784 matched-key
columns for ALL 6272 q rows; the host sums the per-core z partials.  The
only critical-path collective is a single early AllGather of the fp8
q_d (+ q_g) launched right after the q branch, hidden under the k branch.

Dense/global head matmuls run in fp8e4 with DoubleRow (2 contraction rows
per PE cell); weights are pre-scaled x64 on the host, the 1/64 folds into
the activation scale.  End-to-end fp8 rel-err vs the fp32 reference is
~5e-4 (validated in numpy), far under the 2e-2 gate.
"""
import sys

if "/opt/trn_rl_repo" not in sys.path:
    sys.path.insert(0, "/opt/trn_rl_repo")

import numpy as np
import ml_dtypes

import concourse.bass as bass
import concourse.bacc as bacc
import concourse.mybir as mybir
import concourse.tile as tile
from concourse import bass_utils, masks

F8np = ml_dtypes.float8_e4m3     # TRN FP8_EXP4-compatible (bias 7, max 240)
BFnp = ml_dtypes.bfloat16
F32 = mybir.dt.float32
BF16 = mybir.dt.bfloat16
F8 = mybir.dt.float8e4
DR = mybir.MatmulPerfMode.DoubleRow

N_CORES = 8
B, HW, C, D, P, Q = 8, 784, 1024, 2048, 128, 65536
QSH = Q // N_CORES          # 8192 queue rows per core
CT, DT = C // 128, D // 128  # 8, 16
NT = B * HW                 # 6272 total dense rows
RT = NT // 128              # 49 flat q-row tiles
TAU = 0.2
LAM = 0.5
ISC = 1.0 / TAU             # 5.0
AF = mybir.ActivationFunctionType
ALU = mybir.AluOpType

# 784 = 6*128 + 16 partition tiles (correspondence)
PT = [(i * 128, min(128, HW - i * 128)) for i in range(7)]
CH = [(0, 512), (512, HW - 512)]   # free-dim chunks of 784


def _patch_act_tables():
    """Force every activation we use onto the natural_log_exp_and_others
    table set so the kernel needs exactly one ACT_TABLE_LOAD."""
    import concourse.bacc as bacc_mod
    if getattr(bacc_mod, "_act_tables_patched", False):
        return
    from concourse import hw_specs
    orig = hw_specs.get_activation_tables
    ours = {AF.Exp, AF.Ln, AF.Relu, AF.Identity, AF.Copy, AF.Square}
    keep = "natural_log_exp_and_others"

    def patched(arch):
        tabs = orig(arch)
        assert keep in tabs and ours <= tabs[keep]
        return {name: (fns if name == keep else fns - ours)
                for name, fns in tabs.items()}

    bacc_mod.get_activation_tables = patched
    bacc_mod._act_tables_patched = True


def _build(do_compile=True):
    _patch_act_tables()
    nc = bacc.Bacc("TRN2", target_bir_lowering=False, debug=False,
                   num_devices=N_CORES)

    def inp(name, shape, dt):
        return nc.dram_tensor(name, list(shape), dt, kind="ExternalInput")

    env = {}
    env["xq_d"] = inp("xq", (128, CT * HW), F8)    # [c, ct*784+p]
    env["xk_d"] = inp("xk", (128, CT * HW), F8)
    env["wd1_d"] = inp("wd1", (DT // 2, 128, 2 * C), F8)  # dt-pairs, x64
    env["wd1m_d"] = inp("wd1m", (DT // 2, 128, 2 * C), F8)
    env["wd2_d"] = inp("wd2", (128, D), F8)        # [d, dt*128+p] x64
    env["wd2m_d"] = inp("wd2m", (128, D), F8)
    env["wg1_d"] = inp("wg1", (128, CT * D), F8)   # [c, ct*2048+d] x64
    env["wg1m_d"] = inp("wg1m", (128, CT * D), F8)
    env["wg2_d"] = inp("wg2", (128, D), F8)        # like wd2, x64
    env["wg2m_d"] = inp("wg2m", (128, D), F8)
    env["bd1_d"] = inp("bd1", (128, DT), F32)      # [r, dt] = bd1[dt*128+r]
    env["bd1m_d"] = inp("bd1m", (128, DT), F32)
    env["bd2_d"] = inp("bd2", (128, 1), F32)
    env["bd2m_d"] = inp("bd2m", (128, 1), F32)
    env["bg1r_d"] = inp("bg1r", (1, D), BF16)      # bg1 x4096 (bias row)
    env["bg1mr_d"] = inp("bg1mr", (1, D), BF16)
    env["bg2_d"] = inp("bg2", (128, 1), F32)
    env["bg2m_d"] = inp("bg2m", (128, 1), F32)
    env["queueT_d"] = inp("queueT", (128, QSH), F8)  # 8*queue[c0+j, ch]
    env["iota_d"] = inp("iota", (128, 1), F32)
    env["onesc_d"] = inp("onesc", (128, 1), F32)
    env["onesr_d"] = inp("onesr", (1, 128), F32)
    env["ones8_d"] = inp("ones8", (1, 8), BF16)

    env["outz_d"] = nc.dram_tensor("outz", [128, RT], F32,
                                   kind="ExternalOutput")
    env["outs_d"] = nc.dram_tensor("outs", [1, 24], F32,
                                   kind="ExternalOutput")

    with tile.TileContext(nc) as tc:
        with tc.tile_pool(name="dramp", bufs=1, space="DRAM") as dpool:
            env["ag_in"] = dpool.tile([128 * 800], F8, name="ag_in")
            env["ag_out"] = dpool.tile([N_CORES * 128 * 800], F8,
                                       name="ag_out", addr_space="Shared")
            with tc.tile_pool(name="cst", bufs=1) as cst:
                _body(nc, tc, env, cst)
    if do_compile:
        nc.compile()
    return nc


def _dense_branch(nc, tc, env, cst, br, pools, tiles, w1tiles=None):
    """One dense-head branch (q: br=0, k: br=1) -> normalized [128, HW]."""
    g = lambda k: env[k]
    sfx = "" if br == 0 else "m"
    x_sb = tiles["xq8" if br == 0 else "xk8"]
    w1_d = g("wd1" + sfx + "_d")
    w2_sb = tiles["wd2" + sfx]
    b1 = tiles["bd1" + sfx]
    b2 = tiles["bd2" + sfx]
    dst_bf = tiles["qdT_bf" if br == 0 else "kdT_bf"]
    w1p, hp, l2s, ps_h, ps_m = (pools["w1p"], pools["hp"], pools["l2s"],
                                pools["ps_h"], pools["ps_m"])

    xv = x_sb[:].rearrange("c (t p) -> c t p", t=CT)
    qd_ps = ps_m.tile([128, HW], F32, name=f"qd_ps{br}", tag="m")
    hq = None
    hq_prev = None

    def l2_pair(hsrc, dp):
        w2v = w2_sb[:].rearrange("c (t d) -> c t d", t=DT)
        hv = hsrc[:].rearrange("c (j p) -> c j p", j=2)
        for (o, n) in CH:
            nc.tensor.matmul(
                qd_ps[:, o:o + n],
                lhsT=w2v[:, 2 * dp:2 * dp + 2, :],
                rhs=hv[:, :, o:o + n],
                start=(dp == 0), stop=(dp == DT // 2 - 1),
                perf_mode=DR)

    for dt in range(DT):
        # one DMA per dt-pair: 2 KB/partition transfers use the DMA
        # engines far better than 1 KB ones
        if dt % 2 == 0:
            w1t = w1tiles[dt // 2] if w1tiles else w1p.tile(
                [128, 2 * C], F8, name=f"w1t{br}")
            if not w1tiles:
                nc.sync.dma_start(w1t[:], w1_d[dt // 2, :, :])
        h_ps = ps_h.tile([128, HW], F32, name="h_ps", tag="h")
        wv = w1t[:, (dt % 2) * C:(dt % 2 + 1) * C].rearrange(
            "c (t d) -> c t d", t=CT)
        for kp in range(CT // 2):
            for (o, n) in CH:
                nc.tensor.matmul(
                    h_ps[:, o:o + n],
                    lhsT=wv[:, 2 * kp:2 * kp + 2, :],
                    rhs=xv[:, 2 * kp:2 * kp + 2, o:o + n],
                    start=(kp == 0), stop=(kp == CT // 2 - 1),
                    perf_mode=DR)
        # L2 for the pair two dts back: by now its relus have finished, so
        # the PE never stalls waiting on the ACT chain
        if dt % 2 == 0 and dt >= 2:
            l2_pair(hq, dt // 2 - 1)
        if dt % 2 == 0:
            hq = hp.tile([128, 2 * HW], F8, name=f"hq{br}")
        nc.scalar.activation(hq[:, (dt % 2) * HW:(dt % 2 + 1) * HW],
                             h_ps[:], AF.Relu, bias=b1[:, dt:dt + 1],
                             scale=1.0 / 64.0)
        if br == 0:
            # stagger the k-branch / tail input DMAs on the scalar ring so
            # they don't compete with the q-critical loads
            if dt == 2:
                nc.gpsimd.dma_start(tiles["xk8"][:], g("xk_d")[:])
                # pooled sums for the k ghead: must be emitted after the
                # xk8 DMA (program order defines the dependency), runs on
                # the otherwise idle DVE during the q branch
                gsum = tiles["gsum1"]
                for ct in range(CT):
                    pscr = pools["pscr"].tile([128, HW], F8, name="pscr")
                    nc.vector.tensor_scalar(
                        pscr[:], tiles["xk8"][:, ct * HW:(ct + 1) * HW],
                        1.0, None, op0=ALU.mult, op1=ALU.add,
                        accum_out=gsum[:, ct:ct + 1])
                nc.vector.tensor_scalar_mul(
                    tiles["qd8s"][:, HW + 8:HW + 16], gsum[:], 64.0 / HW)

            elif dt == 4:
                nc.scalar.dma_start(tiles["wg1"][:], g("wg1_d")[:])
                nc.scalar.dma_start(tiles["wg2"][:], g("wg2_d")[:])
            elif dt == 8:
                nc.scalar.dma_start(tiles["wd2m"][:], g("wd2m_d")[:])

    l2_pair(hq, DT // 2 - 1)

    # bias + l2 normalize along channels (partition dim)
    qdT_f = l2s.tile([128, HW], F32, name=f"qdT_f{br}")
    nc.scalar.activation(qdT_f[:], qd_ps[:], AF.Identity, bias=b2[:],
                         scale=1.0 / 64.0)
    sq = l2s.tile([128, HW], BF16, name=f"sq{br}")
    nc.scalar.activation(sq[:], qdT_f[:], AF.Square)
    ssq_ps = ps_m.tile([1, HW], F32, name=f"ssq{br}", tag="m")
    for (o, n) in CH:
        nc.tensor.matmul(ssq_ps[:, o:o + n], lhsT=tiles["onescb"][:],
                         rhs=sq[:, o:o + n], start=True, stop=True)
    nrm = l2s.tile([1, HW], F32, name=f"nrm{br}")
    nc.vector.tensor_scalar_max(nrm[:], ssq_ps[:], 1e-12)
    nrm2 = l2s.tile([1, HW], F32, name=f"nrm2{br}")
    nc.scalar.activation(nrm2[:], nrm[:], AF.Ln)
    rn = l2s.tile([1, HW], F32, name=f"rn{br}")
    nc.scalar.activation(rn[:], nrm2[:], AF.Exp, scale=-0.5)
    rnb_ps = ps_m.tile([128, HW], F32, name=f"rnb{br}", tag="m")
    for (o, n) in CH:
        nc.tensor.matmul(rnb_ps[:, o:o + n], lhsT=tiles["onesr"][:],
                         rhs=rn[:, o:o + n], start=True, stop=True)
    nc.vector.tensor_mul(dst_bf[:], qdT_f[:], rnb_ps[:])
    return dst_bf


def _ghead_all(nc, tc, env, cst, br, pools, tiles):
    """Global head for ALL 8 images (q: br=0, k: br=1) from the gathered
    pooled features (g*64, fp8, staged at stride 16 with img innermost)."""
    g = lambda k: env[k]
    sfx = "" if br == 0 else "m"
    w1_sb = tiles["wg1" + sfx]
    w2_sb = tiles["wg2" + sfx]
    b1r = tiles["bg1r" if br == 0 else "bg1mr"]
    b2 = tiles["bg2" + sfx]
    gp, ps_m = pools["gp"], pools["ps_m"]
    ones8 = tiles["ones8"]

    gq_v = tiles[f"gq_all{br}"][:].rearrange("c (t s) -> c t s", s=16)
    # L1: h_g[8, 2048] = (g*64) @ (Wg1*64) / 4096 + bg1, in 1024-chunks
    # (fewer PE<->ACT round trips on this latency-bound chain)
    hgb = gp.tile([8, D], BF16, name=f"hgb{br}")
    w1v = w1_sb[:].rearrange("c (t d) -> c t d", t=CT)
    for ch in range(2):
        hg_ps = ps_m.tile([8, 1024], F32, name=f"hg{br}", tag="m")
        for sub in range(2):
            o = ch * 1024 + sub * 512
            for kp in range(CT // 2):
                nc.tensor.matmul(
                    hg_ps[:, sub * 512:(sub + 1) * 512],
                    lhsT=gq_v[:, 2 * kp:2 * kp + 2, 0:8],
                    rhs=w1v[:, 2 * kp:2 * kp + 2, o:o + 512],
                    start=(kp == 0), stop=False, perf_mode=DR)
            nc.tensor.matmul(hg_ps[:, sub * 512:(sub + 1) * 512],
                             lhsT=ones8[0:1, 0:8],
                             rhs=b1r[0:1, o:o + 512],
                             start=False, stop=True)
        nc.scalar.activation(hgb[0:8, ch * 1024:(ch + 1) * 1024], hg_ps[:],
                             AF.Relu, scale=1.0 / 4096.0)
    # transpose h_g -> [128, dt*8+img] via small K=8 matmuls with identity
    hgt_ps = ps_m.tile([128, DT * 8], F32, name=f"hgt{br}", tag="m")
    for dt in range(DT):
        nc.tensor.matmul(hgt_ps[:, dt * 8:(dt + 1) * 8],
                         lhsT=hgb[0:8, dt * 128:(dt + 1) * 128],
                         rhs=tiles["id_b"][0:8, 0:8],
                         start=(dt == 0), stop=(dt == DT - 1))
    hgt8 = gp.tile([128, DT * 16], F8, name=f"hgt8{br}")
    hgt_v = hgt8[:].rearrange("c (t s) -> c t s", s=16)
    nc.scalar.activation(hgt_v[:, :, 0:8],
                         hgt_ps[:].rearrange("c (t i) -> c t i", i=8),
                         AF.Copy)
    # L2: q_g[128, 8]
    qg_ps = ps_m.tile([128, 8], F32, name=f"qg{br}", tag="m")
    w2v = w2_sb[:].rearrange("c (t d) -> c t d", t=DT)
    for dp in range(DT // 2):
        nc.tensor.matmul(qg_ps[:], lhsT=w2v[:, 2 * dp:2 * dp + 2, :],
                         rhs=hgt_v[:, 2 * dp:2 * dp + 2, 0:8],
                         start=(dp == 0), stop=(dp == DT // 2 - 1),
                         perf_mode=DR)
    qgT_f = gp.tile([128, 8], F32, name=f"qgT_f{br}")
    nc.scalar.activation(qgT_f[:], qg_ps[:], AF.Identity, bias=b2[:],
                         scale=1.0 / 64.0)
    sqg = gp.tile([128, 8], BF16, name=f"sqg{br}")
    nc.scalar.activation(sqg[:], qgT_f[:], AF.Square)
    ssg_ps = ps_m.tile([1, 8], F32, name=f"ssg{br}", tag="m")
    nc.tensor.matmul(ssg_ps[:], lhsT=tiles["onescb"][:], rhs=sqg[:],
                     start=True, stop=True)
    nrg = gp.tile([1, 8], F32, name=f"nrg{br}")
    nc.vector.tensor_scalar_max(nrg[:], ssg_ps[:], 1e-12)
    nrg2 = gp.tile([1, 8], F32, name=f"nrg2{br}")
    nc.scalar.activation(nrg2[:], nrg[:], AF.Ln)
    rng = gp.tile([1, 8], F32, name=f"rng{br}")
    nc.scalar.activation(rng[:], nrg2[:], AF.Exp, scale=-0.5)
    rngb_ps = ps_m.tile([128, 8], F32, name=f"rngb{br}", tag="m")
    nc.tensor.matmul(rngb_ps[:], lhsT=tiles["onesr"][:], rhs=rng[:],
                     start=True, stop=True)
    dst_bf = tiles["qgT_bf" if br == 0 else "kgT_bf"]
    nc.vector.tensor_mul(dst_bf[:], qgT_f[:], rngb_ps[:])
    return dst_bf


def _body(nc, tc, env, cst):
    g = lambda k: env[k]
    tiles = {}

    # ---------------- inputs into SBUF ----------------
    # xq rides the scalar ring (in halves) while the wd1 pairs stream on
    # the sync ring — the two critical loads never serialize each other
    tiles["xq8"] = cst.tile([128, CT * HW], F8, name="xq8")
    half = CT * HW // 2
    nc.scalar.dma_start(tiles["xq8"][:, 0:half], g("xq_d")[:, 0:half])
    nc.scalar.dma_start(tiles["xq8"][:, half:], g("xq_d")[:, half:])
    tiles["wd2"] = cst.tile([128, D], F8, name="wd2")
    nc.scalar.dma_start(tiles["wd2"][:], g("wd2_d")[:])
    tiles["wg2"] = cst.tile([128, D], F8, name="wg2")
    tiles["wg1"] = cst.tile([128, CT * D], F8, name="wg1")
    # k-side tiles (DMAs staggered inside the q loop)
    tiles["xk8"] = cst.tile([128, CT * HW], F8, name="xk8")
    tiles["wd2m"] = cst.tile([128, D], F8, name="wd2m")
    tiles["wg1m"] = cst.tile([128, CT * D], F8, name="wg1m")
    tiles["wg2m"] = cst.tile([128, D], F8, name="wg2m")
    tiles["queueT8"] = cst.tile([128, QSH], F8, name="queueT8")
    # small consts on the gpsimd ring
    for nm, shp, dt in (("iota", (128, 1), F32), ("onesc", (128, 1), F32),
                        ("onesr", (1, 128), F32), ("ones8", (1, 8), BF16),
                        ("bd1", (128, DT), F32), ("bd1m", (128, DT), F32),
                        ("bd2", (128, 1), F32), ("bd2m", (128, 1), F32),
                        ("bg1r", (1, D), BF16), ("bg1mr", (1, D), BF16),
                        ("bg2", (128, 1), F32), ("bg2m", (128, 1), F32)):
        t = cst.tile(list(shp), dt, name=nm)
        nc.gpsimd.dma_start(t[:], g(nm + "_d")[:])
        tiles[nm] = t
    tiles["onescb"] = cst.tile([128, 1], BF16, name="onescb")
    nc.vector.tensor_copy(tiles["onescb"][:], tiles["onesc"][:])
    tiles["onesB"] = cst.tile([128, 128], F32, name="onesB")
    nc.vector.memset(tiles["onesB"][:], 1.0)
    id_f = cst.tile([128, 128], F32, name="id_f")
    masks.make_identity(nc, id_f[:])
    id_b = cst.tile([128, 128], BF16, name="id_b")
    masks.make_identity(nc, id_b[:])

    # long-lived results
    for nm, shp, dt in (("qdT_bf", (128, HW), BF16),
                        ("kdT_bf", (128, HW), BF16),
                        ("qgT_bf", (128, 8), BF16),
                        ("kgT_bf", (128, 8), BF16),
                        ("qd8s", (128, 800), F8),
                        ("qall", (128, NT), F8),
                        ("qg8all", (128, 8), F8),
                        ("gq_all0", (128, CT * 16), F8),
                        ("gq_all1", (128, CT * 16), F8),
                        ("matchT", (128, HW), BF16),
                        ("matchT8", (128, HW), F8),
                        ("zpart", (128, RT), F32),
                        ("fin", (1, 24), F32)):
        tiles[nm] = cst.tile(list(shp), dt, name=nm)
    tiles["id_b"] = id_b
    nc.vector.memset(tiles["fin"][:], 0.0)

    pools = {}
    with tc.tile_pool(name="w1p", bufs=4) as pools["w1p"], \
         tc.tile_pool(name="w1k", bufs=4) as pools["w1k"], \
         tc.tile_pool(name="hp", bufs=2) as pools["hp"], \
         tc.tile_pool(name="l2s", bufs=2) as pools["l2s"], \
         tc.tile_pool(name="gp", bufs=1) as pools["gp"], \
         tc.tile_pool(name="pscr", bufs=2) as pools["pscr"], \
         tc.tile_pool(name="ps_h", bufs=2, space="PSUM") as pools["ps_h"], \
         tc.tile_pool(name="ps_m", bufs=2, space="PSUM") as pools["ps_m"]:

        # pooled feature sums for the q ghead, up front on the idle DVE
        # (the k-side pooling is emitted right after the xk8 DMA below)
        tiles["gsum0"] = cst.tile([128, CT], F32, name="gsum0")
        tiles["gsum1"] = cst.tile([128, CT], F32, name="gsum1")
        for ct in range(CT):
            pscr = pools["pscr"].tile([128, HW], F8, name="pscr")
            nc.vector.tensor_scalar(
                pscr[:], tiles["xq8"][:, ct * HW:(ct + 1) * HW],
                1.0, None, op0=ALU.mult, op1=ALU.add,
                accum_out=tiles["gsum0"][:, ct:ct + 1])
            if ct == 1:
                # gate the next wave of input DMAs behind this point of
                # the DVE stream: a dummy first-writer makes the (otherwise
                # dependency-free) loads wait, so they cannot steal HBM
                # bandwidth from the critical xq/wd1 stream at t=0
                for nm in ("xk8", "wg1", "wg2"):
                    nc.vector.memset(tiles[nm][:, 0:1], 0.0)
        # own pooled features (g*64, f8) ride along in the AllGather
        nc.vector.tensor_scalar_mul(tiles["qd8s"][:, HW:HW + 8],
                                    tiles["gsum0"][:], 64.0 / HW)

        # ========== q branch, then the AllGather ==========
        _dense_branch(nc, tc, env, cst, 0, pools, tiles)
        # prefetch the k-branch W1 pairs right behind the q pairs on the
        # sync ring (ring order keeps them off the critical q stream)
        w1k = []
        for dp in range(DT // 2):
            t = pools["w1k"].tile([128, 2 * C], F8, name="w1k")
            nc.sync.dma_start(t[:], g("wd1m_d")[dp, :, :])
            w1k.append(t)
        nc.vector.tensor_scalar_mul(tiles["qd8s"][:, 0:HW],
                                    tiles["qdT_bf"][:], 8.0)
        ag_in, ag_out = g("ag_in"), g("ag_out")
        nc.gpsimd.dma_start(ag_in[:].rearrange("(c p) -> c p", c=128),
                            tiles["qd8s"][:])
        nc.gpsimd.collective_compute(
            "AllGather", ALU.bypass, replica_groups=[list(range(N_CORES))],
            ins=[ag_in.opt()], outs=[ag_out.opt()])
        # low-urgency loads ride the gpsimd ring behind the AG staging,
        # so they cannot compete with the q/k-critical streams
        nc.gpsimd.dma_start(tiles["wg1m"][:], g("wg1m_d")[:])
        nc.gpsimd.dma_start(tiles["wg2m"][:], g("wg2m_d")[:])
        nc.gpsimd.dma_start(tiles["queueT8"][:], g("queueT_d")[:])

        # ========== k branch ==========
        _dense_branch(nc, tc, env, cst, 1, pools, tiles, w1tiles=w1k)

        # AG-output loads (wait on the collective, nothing else on sync).
        # Pooled features land in stride-16 staging (img innermost); the
        # big qall block is split per rank so the first logits tiles can
        # start as soon as the head of the buffer lands.
        agv = ag_out[:].rearrange("(r c p) -> c r p", r=N_CORES, c=128)
        for br2, off in ((0, HW), (1, HW + 8)):
            for t in range(CT):
                nc.sync.dma_start(
                    tiles[f"gq_all{br2}"][:, t * 16:t * 16 + 8],
                    agv[:, :, off + t])
        for r in range(N_CORES):
            nc.sync.dma_start(
                tiles["qall"][:, r * HW:(r + 1) * HW], agv[:, r, 0:HW])

        # ========== correspondence (own image, bf16) ==========
        qdT_bf, kdT_bf = tiles["qdT_bf"], tiles["kdT_bf"]
        with tc.tile_pool(name="cor", bufs=1) as cor, \
             tc.tile_pool(name="cor2", bufs=2) as cor2:
            sim_sb = cor.tile([128, 7 * HW], BF16, name="sim_sb")
            for i, (po, pn) in enumerate(PT):
                s_ps = pools["ps_h"].tile([128, HW], F32, name="s_ps",
                                          tag="h")
                for (o, n) in CH:
                    nc.tensor.matmul(s_ps[0:pn, o:o + n],
                                     lhsT=qdT_bf[:, po:po + pn],
                                     rhs=kdT_bf[:, o:o + n],
                                     start=True, stop=True)
                nc.scalar.activation(sim_sb[0:pn, i * HW:i * HW + HW],
                                     s_ps[0:pn, :], AF.Copy)
            mx8 = cor.tile([128, 8], F32, name="mx8")
            ix8 = cor.tile([128, 8], mybir.dt.uint32, name="ix8")
            ixf = cor.tile([128, 7], F32, name="ixf")
            for i, (po, pn) in enumerate(PT):
                nc.vector.max(mx8[0:pn, :], sim_sb[0:pn, i * HW:i * HW + HW])
                nc.vector.max_index(ix8[0:pn, :], mx8[0:pn, :],
                                    sim_sb[0:pn, i * HW:i * HW + HW])
                nc.vector.tensor_copy(ixf[0:pn, i:i + 1], ix8[0:pn, 0:1])

            # broadcast the per-pixel argmax indices down the partitions
            ir_sb = cor.tile([1, HW], F32, name="ir_sb")
            for i, (po, pn) in enumerate(PT):
                ir_ps = pools["ps_m"].tile([1, 128], F32, name="ir_ps",
                                           tag="m")
                nc.tensor.transpose(ir_ps[0:1, 0:pn], ixf[0:pn, i:i + 1],
                                    id_f[0:pn, 0:pn])
                nc.scalar.activation(ir_sb[0:1, po:po + pn],
                                     ir_ps[0:1, 0:pn], AF.Copy)
            ib_ps = pools["ps_m"].tile([128, HW], F32, name="ib_ps", tag="m")
            for (o, n) in CH:
                nc.tensor.matmul(ib_ps[:, o:o + n], lhsT=tiles["onesr"][:],
                                 rhs=ir_sb[:, o:o + n], start=True, stop=True)
            ib_sb = cor.tile([128, HW], F32, name="ib_sb")
            nc.scalar.activation(ib_sb[:], ib_ps[:], AF.Copy)
            # gather matched keys via one-hot matmuls; mt_ps stays resident
            # in ps_m while kt transposes rotate through ps_h
            mt_ps = pools["ps_m"].tile([128, HW], F32, name="mt_ps", tag="m")
            for i, (po, pn) in enumerate(PT):
                S = cor2.tile([128, HW], BF16, name="S")
                nc.vector.tensor_scalar(
                    S[0:pn, :], ib_sb[0:pn, :], tiles["iota"][0:pn, :],
                    float(po), op0=ALU.subtract, op1=ALU.is_equal)
                kt_ps = pools["ps_h"].tile([128, 128], BF16, name="kt_ps",
                                           tag="h")
                nc.tensor.transpose(kt_ps[0:pn, :], kdT_bf[:, po:po + pn],
                                    id_b[:, :])
                kt_sb = cor2.tile([128, 128], BF16, name="kt_sb")
                nc.scalar.activation(kt_sb[0:pn, :], kt_ps[0:pn, :], AF.Copy)
                for (o, n) in CH:
                    nc.tensor.matmul(mt_ps[:, o:o + n], lhsT=kt_sb[0:pn, :],
                                     rhs=S[0:pn, o:o + n],
                                     start=(i == 0), stop=(i == 6))
            nc.scalar.activation(tiles["matchT"][:], mt_ps[:], AF.Copy)
            nc.vector.tensor_scalar_mul(tiles["matchT8"][:], mt_ps[:], 8.0)

            # positives: diag = qd . matched (own rows), summed
            posm = cor.tile([128, HW], F32, name="posm")
            nc.vector.tensor_mul(posm[:], qdT_bf[:], tiles["matchT"][:])
            pos_ps = pools["ps_m"].tile([1, HW], F32, name="pos_ps", tag="m")
            for (o, n) in CH:
                nc.tensor.matmul(pos_ps[:, o:o + n], lhsT=tiles["onesc"][:],
                                 rhs=posm[:, o:o + n], start=True, stop=True)
            nc.vector.reduce_sum(tiles["fin"][0:1, 0:1], pos_ps[:],
                                 axis=mybir.AxisListType.X)

        # ========== gathered q: dense logits (ACT-bound tail) ==========
        with tc.tile_pool(name="escr", bufs=3) as escr:
            # dense logits, column shard: all 6272 q rows x own 784 keys;
            # per-row exp sums via DVE (keeps the ACT chain pure Exp).
            # Everything that depends on the AllGather but not on the
            # correspondence — global heads, lpos, queue negatives — is
            # interleaved into this loop so its latency chains resolve
            # under the exp stream.
            qe_sb = tiles["qe_sb"] = cst.tile([128, 512], BF16, name="qe_sb")

            def qe_group(grp):
                qe_ps = pools["ps_m"].tile([128, 64], F32, name="qe_ps",
                                           tag="m")
                for j in range(8):
                    qt = grp * 8 + j
                    nc.tensor.matmul(
                        qe_ps[:, j * 8:(j + 1) * 8],
                        lhsT=tiles["queueT8"][:, qt * 128:(qt + 1) * 128],
                        rhs=tiles["qg8all"][:], start=(j == 0), stop=(j == 7))
                nc.scalar.activation(qe_sb[:, grp * 64:(grp + 1) * 64],
                                     qe_ps[:], AF.Exp, scale=ISC / 64.0)

            for t in range(RT):
                lg_ps = pools["ps_h"].tile([128, HW], F32, name="lg_ps",
                                           tag="h")
                for (o, n) in CH:
                    nc.tensor.matmul(
                        lg_ps[:, o:o + n],
                        lhsT=tiles["qall"][:, t * 128:(t + 1) * 128],
                        rhs=tiles["matchT8"][:, o:o + n],
                        start=True, stop=True)
                es = escr.tile([128, HW], BF16, name="es")
                nc.scalar.activation(es[:], lg_ps[:], AF.Exp,
                                     scale=ISC / 64.0)
                nc.vector.reduce_sum(tiles["zpart"][:, t:t + 1], es[:],
                                     axis=mybir.AxisListType.X)
                if t == 2:
                    _ghead_all(nc, tc, env, cst, 0, pools, tiles)
                    nc.vector.tensor_scalar_mul(tiles["qg8all"][:],
                                                tiles["qgT_bf"][:], 8.0)
                elif t == 8:
                    _ghead_all(nc, tc, env, cst, 1, pools, tiles)
                    lpm = pools["gp"].tile([128, 8], F32, name="lpm")
                    nc.vector.tensor_mul(lpm[:], tiles["qgT_bf"][:],
                                         tiles["kgT_bf"][:])
                    lp_ps = pools["ps_m"].tile([1, 8], F32, name="lp_ps",
                                               tag="m")
                    nc.tensor.matmul(lp_ps[:], lhsT=tiles["onesc"][:],
                                     rhs=lpm[:], start=True, stop=True)
                    nc.vector.tensor_copy(tiles["fin"][0:1, 1:9], lp_ps[:])
                elif t >= 14 and (t - 14) % 4 == 0 and (t - 14) // 4 < 8:
                    qe_group((t - 14) // 4)
            qs_ps = pools["ps_m"].tile([1, 512], F32, name="qs_ps", tag="m")
            nc.tensor.matmul(qs_ps[:], lhsT=tiles["onescb"][:],
                             rhs=tiles["qe_sb"][:], start=True, stop=True)
            nc.vector.reduce_sum(tiles["fin"][0:1, 9:17],
                                 qs_ps[:].rearrange("p (t i) -> p i t", i=8),
                                 axis=mybir.AxisListType.X)

        nc.sync.dma_start(g("outz_d")[:], tiles["zpart"][:])
        nc.sync.dma_start(g("outs_d")[:], tiles["fin"][:])


def _prep_inputs(inputs):
    fq = np.asarray(inputs["feat_q"], np.float32).reshape(B, HW, C)
    fk = np.asarray(inputs["feat_k"], np.float32).reshape(B, HW, C)

    def xT8(x):  # (784, 1024) -> (128, 8*784) f8 with [c, ct*784+p]
        return np.ascontiguousarray(
            x.reshape(HW, CT, 128).transpose(2, 1, 0).reshape(128, CT * HW)
        ).astype(F8np)

    def w1tile(w):  # (1024, 2048) -> (8, 128, 2048) f8 x64, dt-pair major
        t = (w * 64.0).reshape(CT, 128, DT, 128).transpose(2, 1, 0, 3)
        t = t.reshape(DT // 2, 2, 128, C).transpose(0, 2, 1, 3)
        return np.ascontiguousarray(t.reshape(DT // 2, 128, 2 * C)
                                    ).astype(F8np)

    def w2tile(w):  # (2048, 128) -> (128, 2048) f8 x64
        return np.ascontiguousarray(
            (w * 64.0).reshape(DT, 128, 128).transpose(1, 0, 2)
            .reshape(128, D)).astype(F8np)

    def wg1tile(w):  # (1024, 2048) -> (128, 8*2048) f8 x64
        return np.ascontiguousarray(
            (w * 64.0).reshape(CT, 128, D).transpose(1, 0, 2)
            .reshape(128, CT * D)).astype(F8np)

    shared = {
        "wd1": w1tile(inputs["Wd1"]), "wd1m": w1tile(inputs["mWd1"]),
        "wd2": w2tile(inputs["Wd2"]), "wd2m": w2tile(inputs["mWd2"]),
        "wg1": wg1tile(inputs["Wg1"]), "wg1m": wg1tile(inputs["mWg1"]),
        "wg2": w2tile(inputs["Wg2"]), "wg2m": w2tile(inputs["mWg2"]),
        "bd1": np.ascontiguousarray(
            np.asarray(inputs["bd1"], np.float32).reshape(DT, 128).T),
        "bd1m": np.ascontiguousarray(
            np.asarray(inputs["mbd1"], np.float32).reshape(DT, 128).T),
        "bd2": np.asarray(inputs["bd2"], np.float32).reshape(128, 1),
        "bd2m": np.asarray(inputs["mbd2"], np.float32).reshape(128, 1),
        "bg1r": (np.asarray(inputs["bg1"], np.float32) * 4096.0
                 ).reshape(1, D).astype(BFnp),
        "bg1mr": (np.asarray(inputs["mbg1"], np.float32) * 4096.0
                  ).reshape(1, D).astype(BFnp),
        "bg2": np.asarray(inputs["bg2"], np.float32).reshape(128, 1),
        "bg2m": np.asarray(inputs["mbg2"], np.float32).reshape(128, 1),
        "iota": np.arange(128, dtype=np.float32).reshape(128, 1),
        "onesc": np.ones((128, 1), np.float32),
        "onesr": np.ones((1, 128), np.float32),
        "ones8": np.ones((1, 8), np.float32).astype(BFnp),
    }
    queue = np.asarray(inputs["queue"], np.float32)
    in_maps = []
    for c in range(N_CORES):
        m = dict(shared)
        m["xq"] = xT8(fq[c])
        m["xk"] = xT8(fk[c])
        m["queueT"] = np.ascontiguousarray(
            (queue[c * QSH:(c + 1) * QSH] * 8.0).T).astype(F8np)
        in_maps.append(m)
    return in_maps


_NC = None


def _get_nc():
    global _NC
    if _NC is None:
        _NC = _build()
    return _NC


def _host_combine(outz, outs):
    """outz: [8][128, 49] z-partials; outs: [8][1, 24] scalars.

    outs slots: [0] sum(qd.matched) over own rows, [1:9] lpos per image
    (replicated on every core), [9:17] partial sum(exp(l_neg/tau)) per
    image over the core's queue shard.  Dense z row r=t*128+p lives at
    outz[:, p, t].
    """
    outz = np.asarray(outz, np.float64)   # [8, 128, 49]
    outs = np.asarray(outs, np.float64)   # [8, 24]
    z = outz.sum(axis=0)                  # [128, 49]
    zrows = z.T.reshape(-1)               # row r = t*128+p
    pos_total = outs[:, 0].sum()
    l_d = (np.log(zrows).sum() - ISC * pos_total) / NT
    zq = outs[:, 9:17].sum(axis=0)        # [8]
    lpos = outs[0, 1:9]                   # replicated
    lse = np.log(zq + np.exp(ISC * lpos))
    l_g = np.mean(lse - ISC * lpos)
    return np.float32((1.0 - LAM) * l_g + LAM * l_d).reshape(())


def kernel(**inputs) -> np.ndarray:
    nc = _get_nc()
    in_maps = _prep_inputs(inputs)
    res = bass_utils.run_bass_kernel_spmd(nc, in_maps,
                                          core_ids=list(range(N_CORES)))
    outz = np.stack([res.results[c]["outz"] for c in range(N_CORES)])
    outs = np.stack([res.results[c]["outs"].reshape(24)
                     for c in range(N_CORES)])
    return _host_combine(outz, outs)


# revision 68
# speedup vs baseline: 1.3711x; 1.0201x over previous
"""DenseCL loss kernel for 8 TRN2 NeuronCores (v2: fp8 DoubleRow + column-
sharded dense-InfoNCE logits).

Sharding: core c owns image c (dense head + correspondence + matched keys),
queue rows [c*8192, (c+1)*8192), and the COLUMN shard of the flat dense
logits: core c computes partial exp-sums over its own 784 matched-key
columns for ALL 6272 q rows; the host sums the per-core z partials.  The
only critical-path collective is a single early AllGather of the fp8
q_d (+ q_g) launched right after the q branch, hidden under the k branch.

Dense/global head matmuls run in fp8e4 with DoubleRow (2 contraction rows
per PE cell); weights are pre-scaled x64 on the host, the 1/64 folds into
the activation scale.  End-to-end fp8 rel-err vs the fp32 reference is
~5e-4 (validated in numpy), far under the 2e-2 gate.
"""
import sys

if "/opt/trn_rl_repo" not in sys.path:
    sys.path.insert(0, "/opt/trn_rl_repo")

import numpy as np
import ml_dtypes

import concourse.bass as bass
import concourse.bacc as bacc
import concourse.mybir as mybir
import concourse.tile as tile
from concourse import bass_utils, masks

F8np = ml_dtypes.float8_e4m3     # TRN FP8_EXP4-compatible (bias 7, max 240)
BFnp = ml_dtypes.bfloat16
F32 = mybir.dt.float32
BF16 = mybir.dt.bfloat16
F8 = mybir.dt.float8e4
DR = mybir.MatmulPerfMode.DoubleRow

N_CORES = 8
B, HW, C, D, P, Q = 8, 784, 1024, 2048, 128, 65536
QSH = Q // N_CORES          # 8192 queue rows per core
CT, DT = C // 128, D // 128  # 8, 16
NT = B * HW                 # 6272 total dense rows
RT = NT // 128              # 49 flat q-row tiles
TAU = 0.2
LAM = 0.5
ISC = 1.0 / TAU             # 5.0
AF = mybir.ActivationFunctionType
ALU = mybir.AluOpType

# 784 = 6*128 + 16 partition tiles (correspondence)
PT = [(i * 128, min(128, HW - i * 128)) for i in range(7)]
CH = [(0, 512), (512, HW - 512)]   # free-dim chunks of 784


def _patch_act_tables():
    """Force every activation we use onto the natural_log_exp_and_others
    table set so the kernel needs exactly one ACT_TABLE_LOAD."""
    import concourse.bacc as bacc_mod
    if getattr(bacc_mod, "_act_tables_patched", False):
        return
    from concourse import hw_specs
    orig = hw_specs.get_activation_tables
    ours = {AF.Exp, AF.Ln, AF.Relu, AF.Identity, AF.Copy, AF.Square}
    keep = "natural_log_exp_and_others"

    def patched(arch):
        tabs = orig(arch)
        assert keep in tabs and ours <= tabs[keep]
        return {name: (fns if name == keep else fns - ours)
                for name, fns in tabs.items()}

    bacc_mod.get_activation_tables = patched
    bacc_mod._act_tables_patched = True


def _build(do_compile=True):
    _patch_act_tables()
    nc = bacc.Bacc("TRN2", target_bir_lowering=False, debug=False,
                   num_devices=N_CORES)

    def inp(name, shape, dt):
        return nc.dram_tensor(name, list(shape), dt, kind="ExternalInput")

    env = {}
    env["xq_d"] = inp("xq", (128, CT * HW), F8)    # [c, ct*784+p]
    env["xk_d"] = inp("xk", (128, CT * HW), F8)
    env["wd1_d"] = inp("wd1", (DT // 2, 128, 2 * C), F8)  # dt-pairs, x64
    env["wd1m_d"] = inp("wd1m", (DT // 2, 128, 2 * C), F8)
    env["wd2_d"] = inp("wd2", (128, D), F8)        # [d, dt*128+p] x64
    env["wd2m_d"] = inp("wd2m", (128, D), F8)
    env["wg1_d"] = inp("wg1", (128, CT * D), F8)   # [c, ct*2048+d] x64
    env["wg1m_d"] = inp("wg1m", (128, CT * D), F8)
    env["wg2_d"] = inp("wg2", (128, D), F8)        # like wd2, x64
    env["wg2m_d"] = inp("wg2m", (128, D), F8)
    env["bd1_d"] = inp("bd1", (128, DT), F32)      # [r, dt] = bd1[dt*128+r]
    env["bd1m_d"] = inp("bd1m", (128, DT), F32)
    env["bd2_d"] = inp("bd2", (128, 1), F32)
    env["bd2m_d"] = inp("bd2m", (128, 1), F32)
    env["bg1r_d"] = inp("bg1r", (1, D), BF16)      # bg1 x4096 (bias row)
    env["bg1mr_d"] = inp("bg1mr", (1, D), BF16)
    env["bg2_d"] = inp("bg2", (128, 1), F32)
    env["bg2m_d"] = inp("bg2m", (128, 1), F32)
    env["queueT_d"] = inp("queueT", (128, QSH), F8)  # 8*queue[c0+j, ch]
    env["iota_d"] = inp("iota", (128, 1), F32)
    env["onesc_d"] = inp("onesc", (128, 1), F32)
    env["onesr_d"] = inp("onesr", (1, 128), F32)
    env["ones8_d"] = inp("ones8", (1, 8), BF16)

    env["outz_d"] = nc.dram_tensor("outz", [128, RT], F32,
                                   kind="ExternalOutput")
    env["outs_d"] = nc.dram_tensor("outs", [1, 24], F32,
                                   kind="ExternalOutput")

    with tile.TileContext(nc) as tc:
        with tc.tile_pool(name="dramp", bufs=1, space="DRAM") as dpool:
            env["ag_in"] = dpool.tile([128 * 800], F8, name="ag_in")
            env["ag_out"] = dpool.tile([N_CORES * 128 * 800], F8,
                                       name="ag_out", addr_space="Shared")
            with tc.tile_pool(name="cst", bufs=1) as cst:
                _body(nc, tc, env, cst)
    if do_compile:
        nc.compile()
    return nc


def _dense_branch(nc, tc, env, cst, br, pools, tiles, w1tiles=None):
    """One dense-head branch (q: br=0, k: br=1) -> normalized [128, HW]."""
    g = lambda k: env[k]
    sfx = "" if br == 0 else "m"
    x_sb = tiles["xq8" if br == 0 else "xk8"]
    w1_d = g("wd1" + sfx + "_d")
    w2_sb = tiles["wd2" + sfx]
    b1 = tiles["bd1" + sfx]
    b2 = tiles["bd2" + sfx]
    dst_bf = tiles["qdT_bf" if br == 0 else "kdT_bf"]
    w1p, hp, l2s, ps_h, ps_m = (pools["w1p"], pools["hp"], pools["l2s"],
                                pools["ps_h"], pools["ps_m"])

    xv = x_sb[:].rearrange("c (t p) -> c t p", t=CT)
    qd_ps = ps_m.tile([128, HW], F32, name=f"qd_ps{br}", tag="m")
    hq = None
    hq_prev = None

    def l2_pair(hsrc, dp):
        w2v = w2_sb[:].rearrange("c (t d) -> c t d", t=DT)
        hv = hsrc[:].rearrange("c (j p) -> c j p", j=2)
        for (o, n) in CH:
            nc.tensor.matmul(
                qd_ps[:, o:o + n],
                lhsT=w2v[:, 2 * dp:2 * dp + 2, :],
                rhs=hv[:, :, o:o + n],
                start=(dp == 0), stop=(dp == DT // 2 - 1),
                perf_mode=DR)

    for dt in range(DT):
        # one DMA per dt-pair: 2 KB/partition transfers use the DMA
        # engines far better than 1 KB ones
        if dt % 2 == 0:
            w1t = w1tiles[dt // 2] if w1tiles else w1p.tile(
                [128, 2 * C], F8, name=f"w1t{br}")
            if not w1tiles:
                nc.sync.dma_start(w1t[:], w1_d[dt // 2, :, :])
        h_ps = ps_h.tile([128, HW], F32, name="h_ps", tag="h")
        wv = w1t[:, (dt % 2) * C:(dt % 2 + 1) * C].rearrange(
            "c (t d) -> c t d", t=CT)
        for kp in range(CT // 2):
            for (o, n) in CH:
                nc.tensor.matmul(
                    h_ps[:, o:o + n],
                    lhsT=wv[:, 2 * kp:2 * kp + 2, :],
                    rhs=xv[:, 2 * kp:2 * kp + 2, o:o + n],
                    start=(kp == 0), stop=(kp == CT // 2 - 1),
                    perf_mode=DR)
        # L2 for the pair two dts back: by now its relus have finished, so
        # the PE never stalls waiting on the ACT chain
        if dt % 2 == 0 and dt >= 2:
            l2_pair(hq, dt // 2 - 1)
        if dt % 2 == 0:
            hq = hp.tile([128, 2 * HW], F8, name=f"hq{br}")
        nc.scalar.activation(hq[:, (dt % 2) * HW:(dt % 2 + 1) * HW],
                             h_ps[:], AF.Relu, bias=b1[:, dt:dt + 1],
                             scale=1.0 / 64.0)
        if br == 0:
            # stagger the k-branch / tail input DMAs on the scalar ring so
            # they don't compete with the q-critical loads
            if dt == 2:
                nc.gpsimd.dma_start(tiles["xk8"][:], g("xk_d")[:])
                # pooled sums for the k ghead: must be emitted after the
                # xk8 DMA (program order defines the dependency), runs on
                # the otherwise idle DVE during the q branch
                gsum = tiles["gsum1"]
                for ct in range(CT):
                    pscr = pools["pscr"].tile([128, HW], F8, name="pscr")
                    nc.vector.tensor_scalar(
                        pscr[:], tiles["xk8"][:, ct * HW:(ct + 1) * HW],
                        1.0, None, op0=ALU.mult, op1=ALU.add,
                        accum_out=gsum[:, ct:ct + 1])
                nc.vector.tensor_scalar_mul(
                    tiles["qd8s"][:, HW + 8:HW + 16], gsum[:], 64.0 / HW)

            elif dt == 4:
                nc.scalar.dma_start(tiles["wg1"][:], g("wg1_d")[:])
                nc.scalar.dma_start(tiles["wg2"][:], g("wg2_d")[:])
            elif dt == 8:
                nc.scalar.dma_start(tiles["wd2m"][:], g("wd2m_d")[:])

    l2_pair(hq, DT // 2 - 1)

    # bias + l2 normalize along channels (partition dim)
    qdT_f = l2s.tile([128, HW], F32, name=f"qdT_f{br}")
    nc.scalar.activation(qdT_f[:], qd_ps[:], AF.Identity, bias=b2[:],
                         scale=1.0 / 64.0)
    sq = l2s.tile([128, HW], BF16, name=f"sq{br}")
    nc.scalar.activation(sq[:], qdT_f[:], AF.Square)
    ssq_ps = ps_m.tile([1, HW], F32, name=f"ssq{br}", tag="m")
    for (o, n) in CH:
        nc.tensor.matmul(ssq_ps[:, o:o + n], lhsT=tiles["onescb"][:],
                         rhs=sq[:, o:o + n], start=True, stop=True)
    nrm = l2s.tile([1, HW], F32, name=f"nrm{br}")
    nc.vector.tensor_scalar_max(nrm[:], ssq_ps[:], 1e-12)
    nrm2 = l2s.tile([1, HW], F32, name=f"nrm2{br}")
    nc.scalar.activation(nrm2[:], nrm[:], AF.Ln)
    rn = l2s.tile([1, HW], F32, name=f"rn{br}")
    nc.scalar.activation(rn[:], nrm2[:], AF.Exp, scale=-0.5)
    rnb_ps = ps_m.tile([128, HW], F32, name=f"rnb{br}", tag="m")
    for (o, n) in CH:
        nc.tensor.matmul(rnb_ps[:, o:o + n], lhsT=tiles["onesr"][:],
                         rhs=rn[:, o:o + n], start=True, stop=True)
    nc.vector.tensor_mul(dst_bf[:], qdT_f[:], rnb_ps[:])
    return dst_bf


def _ghead_all(nc, tc, env, cst, br, pools, tiles):
    """Global head for ALL 8 images (q: br=0, k: br=1) from the gathered
    pooled features (g*64, fp8, staged at stride 16 with img innermost)."""
    g = lambda k: env[k]
    sfx = "" if br == 0 else "m"
    w1_sb = tiles["wg1" + sfx]
    w2_sb = tiles["wg2" + sfx]
    b1r = tiles["bg1r" if br == 0 else "bg1mr"]
    b2 = tiles["bg2" + sfx]
    gp, ps_m = pools["gp"], pools["ps_m"]
    ones8 = tiles["ones8"]

    gq_v = tiles[f"gq_all{br}"][:].rearrange("c (t s) -> c t s", s=16)
    # L1: h_g[8, 2048] = (g*64) @ (Wg1*64) / 4096 + bg1, in 1024-chunks
    # (fewer PE<->ACT round trips on this latency-bound chain)
    hgb = gp.tile([8, D], BF16, name=f"hgb{br}")
    w1v = w1_sb[:].rearrange("c (t d) -> c t d", t=CT)
    for ch in range(2):
        hg_ps = ps_m.tile([8, 1024], F32, name=f"hg{br}", tag="m")
        for sub in range(2):
            o = ch * 1024 + sub * 512
            for kp in range(CT // 2):
                nc.tensor.matmul(
                    hg_ps[:, sub * 512:(sub + 1) * 512],
                    lhsT=gq_v[:, 2 * kp:2 * kp + 2, 0:8],
                    rhs=w1v[:, 2 * kp:2 * kp + 2, o:o + 512],
                    start=(kp == 0), stop=False, perf_mode=DR)
            nc.tensor.matmul(hg_ps[:, sub * 512:(sub + 1) * 512],
                             lhsT=ones8[0:1, 0:8],
                             rhs=b1r[0:1, o:o + 512],
                             start=False, stop=True)
        # relu on the DVE: keeps the ACT chain free for the logits exps
        nc.vector.tensor_scalar(hgb[0:8, ch * 1024:(ch + 1) * 1024],
                                hg_ps[:], 1.0 / 4096.0, 0.0,
                                op0=ALU.mult, op1=ALU.max)
    # transpose h_g -> [128, dt*8+img] via small K=8 matmuls with identity
    hgt_ps = ps_m.tile([128, DT * 8], F32, name=f"hgt{br}", tag="m")
    for dt in range(DT):
        nc.tensor.matmul(hgt_ps[:, dt * 8:(dt + 1) * 8],
                         lhsT=hgb[0:8, dt * 128:(dt + 1) * 128],
                         rhs=tiles["id_b"][0:8, 0:8],
                         start=(dt == 0), stop=(dt == DT - 1))
    hgt8 = gp.tile([128, DT * 16], F8, name=f"hgt8{br}")
    hgt_v = hgt8[:].rearrange("c (t s) -> c t s", s=16)
    nc.vector.tensor_copy(hgt_v[:, :, 0:8],
                          hgt_ps[:].rearrange("c (t i) -> c t i", i=8))
    # L2: q_g[128, 8]
    qg_ps = ps_m.tile([128, 8], F32, name=f"qg{br}", tag="m")
    w2v = w2_sb[:].rearrange("c (t d) -> c t d", t=DT)
    for dp in range(DT // 2):
        nc.tensor.matmul(qg_ps[:], lhsT=w2v[:, 2 * dp:2 * dp + 2, :],
                         rhs=hgt_v[:, 2 * dp:2 * dp + 2, 0:8],
                         start=(dp == 0), stop=(dp == DT // 2 - 1),
                         perf_mode=DR)
    qgT_f = gp.tile([128, 8], F32, name=f"qgT_f{br}")
    nc.scalar.activation(qgT_f[:], qg_ps[:], AF.Identity, bias=b2[:],
                         scale=1.0 / 64.0)
    sqg = gp.tile([128, 8], BF16, name=f"sqg{br}")
    nc.scalar.activation(sqg[:], qgT_f[:], AF.Square)
    ssg_ps = ps_m.tile([1, 8], F32, name=f"ssg{br}", tag="m")
    nc.tensor.matmul(ssg_ps[:], lhsT=tiles["onescb"][:], rhs=sqg[:],
                     start=True, stop=True)
    nrg = gp.tile([1, 8], F32, name=f"nrg{br}")
    nc.vector.tensor_scalar_max(nrg[:], ssg_ps[:], 1e-12)
    nrg2 = gp.tile([1, 8], F32, name=f"nrg2{br}")
    nc.scalar.activation(nrg2[:], nrg[:], AF.Ln)
    rng = gp.tile([1, 8], F32, name=f"rng{br}")
    nc.scalar.activation(rng[:], nrg2[:], AF.Exp, scale=-0.5)
    rngb_ps = ps_m.tile([128, 8], F32, name=f"rngb{br}", tag="m")
    nc.tensor.matmul(rngb_ps[:], lhsT=tiles["onesr"][:], rhs=rng[:],
                     start=True, stop=True)
    dst_bf = tiles["qgT_bf" if br == 0 else "kgT_bf"]
    nc.vector.tensor_mul(dst_bf[:], qgT_f[:], rngb_ps[:])
    return dst_bf


def _body(nc, tc, env, cst):
    g = lambda k: env[k]
    tiles = {}

    # ---------------- inputs into SBUF ----------------
    # xq rides the scalar ring (in halves) while the wd1 pairs stream on
    # the sync ring — the two critical loads never serialize each other
    tiles["xq8"] = cst.tile([128, CT * HW], F8, name="xq8")
    half = CT * HW // 2
    nc.scalar.dma_start(tiles["xq8"][:, 0:half], g("xq_d")[:, 0:half])
    nc.scalar.dma_start(tiles["xq8"][:, half:], g("xq_d")[:, half:])
    tiles["wd2"] = cst.tile([128, D], F8, name="wd2")
    nc.scalar.dma_start(tiles["wd2"][:], g("wd2_d")[:])
    tiles["wg2"] = cst.tile([128, D], F8, name="wg2")
    tiles["wg1"] = cst.tile([128, CT * D], F8, name="wg1")
    # k-side tiles (DMAs staggered inside the q loop)
    tiles["xk8"] = cst.tile([128, CT * HW], F8, name="xk8")
    tiles["wd2m"] = cst.tile([128, D], F8, name="wd2m")
    tiles["wg1m"] = cst.tile([128, CT * D], F8, name="wg1m")
    tiles["wg2m"] = cst.tile([128, D], F8, name="wg2m")
    tiles["queueT8"] = cst.tile([128, QSH], F8, name="queueT8")
    # small consts on the gpsimd ring
    for nm, shp, dt in (("iota", (128, 1), F32), ("onesc", (128, 1), F32),
                        ("onesr", (1, 128), F32), ("ones8", (1, 8), BF16),
                        ("bd1", (128, DT), F32), ("bd1m", (128, DT), F32),
                        ("bd2", (128, 1), F32), ("bd2m", (128, 1), F32),
                        ("bg1r", (1, D), BF16), ("bg1mr", (1, D), BF16),
                        ("bg2", (128, 1), F32), ("bg2m", (128, 1), F32)):
        t = cst.tile(list(shp), dt, name=nm)
        nc.gpsimd.dma_start(t[:], g(nm + "_d")[:])
        tiles[nm] = t
    tiles["onescb"] = cst.tile([128, 1], BF16, name="onescb")
    nc.vector.tensor_copy(tiles["onescb"][:], tiles["onesc"][:])
    tiles["onesB"] = cst.tile([128, 128], F32, name="onesB")
    nc.vector.memset(tiles["onesB"][:], 1.0)
    id_f = cst.tile([128, 128], F32, name="id_f")
    masks.make_identity(nc, id_f[:])
    id_b = cst.tile([128, 128], BF16, name="id_b")
    masks.make_identity(nc, id_b[:])

    # long-lived results
    for nm, shp, dt in (("qdT_bf", (128, HW), BF16),
                        ("kdT_bf", (128, HW), BF16),
                        ("qgT_bf", (128, 8), BF16),
                        ("kgT_bf", (128, 8), BF16),
                        ("qd8s", (128, 800), F8),
                        ("qall", (128, NT), F8),
                        ("qg8all", (128, 8), F8),
                        ("gq_all0", (128, CT * 16), F8),
                        ("gq_all1", (128, CT * 16), F8),
                        ("matchT", (128, HW), BF16),
                        ("matchT8", (128, HW), F8),
                        ("zpart", (128, RT), F32),
                        ("fin", (1, 24), F32)):
        tiles[nm] = cst.tile(list(shp), dt, name=nm)
    tiles["id_b"] = id_b
    nc.vector.memset(tiles["fin"][:], 0.0)

    pools = {}
    with tc.tile_pool(name="w1p", bufs=4) as pools["w1p"], \
         tc.tile_pool(name="w1k", bufs=4) as pools["w1k"], \
         tc.tile_pool(name="hp", bufs=2) as pools["hp"], \
         tc.tile_pool(name="l2s", bufs=2) as pools["l2s"], \
         tc.tile_pool(name="gp", bufs=1) as pools["gp"], \
         tc.tile_pool(name="pscr", bufs=2) as pools["pscr"], \
         tc.tile_pool(name="ps_h", bufs=2, space="PSUM") as pools["ps_h"], \
         tc.tile_pool(name="ps_m", bufs=2, space="PSUM") as pools["ps_m"]:

        # pooled feature sums for the q ghead, up front on the idle DVE
        # (the k-side pooling is emitted right after the xk8 DMA below)
        tiles["gsum0"] = cst.tile([128, CT], F32, name="gsum0")
        tiles["gsum1"] = cst.tile([128, CT], F32, name="gsum1")
        for ct in range(CT):
            pscr = pools["pscr"].tile([128, HW], F8, name="pscr")
            nc.vector.tensor_scalar(
                pscr[:], tiles["xq8"][:, ct * HW:(ct + 1) * HW],
                1.0, None, op0=ALU.mult, op1=ALU.add,
                accum_out=tiles["gsum0"][:, ct:ct + 1])
            if ct == 1:
                # gate the next wave of input DMAs behind this point of
                # the DVE stream: a dummy first-writer makes the (otherwise
                # dependency-free) loads wait, so they cannot steal HBM
                # bandwidth from the critical xq/wd1 stream at t=0
                for nm in ("xk8", "wg1", "wg2"):
                    nc.vector.memset(tiles[nm][:, 0:1], 0.0)
        # own pooled features (g*64, f8) ride along in the AllGather
        nc.vector.tensor_scalar_mul(tiles["qd8s"][:, HW:HW + 8],
                                    tiles["gsum0"][:], 64.0 / HW)

        # ========== q branch, then the AllGather ==========
        _dense_branch(nc, tc, env, cst, 0, pools, tiles)
        # prefetch the k-branch W1 pairs right behind the q pairs on the
        # sync ring (ring order keeps them off the critical q stream)
        w1k = []
        for dp in range(DT // 2):
            t = pools["w1k"].tile([128, 2 * C], F8, name="w1k")
            nc.sync.dma_start(t[:], g("wd1m_d")[dp, :, :])
            w1k.append(t)
        nc.vector.tensor_scalar_mul(tiles["qd8s"][:, 0:HW],
                                    tiles["qdT_bf"][:], 8.0)
        ag_in, ag_out = g("ag_in"), g("ag_out")
        nc.gpsimd.dma_start(ag_in[:].rearrange("(c p) -> c p", c=128),
                            tiles["qd8s"][:])
        nc.gpsimd.collective_compute(
            "AllGather", ALU.bypass, replica_groups=[list(range(N_CORES))],
            ins=[ag_in.opt()], outs=[ag_out.opt()])
        # low-urgency loads ride the gpsimd ring behind the AG staging,
        # so they cannot compete with the q/k-critical streams
        nc.gpsimd.dma_start(tiles["wg1m"][:], g("wg1m_d")[:])
        nc.gpsimd.dma_start(tiles["wg2m"][:], g("wg2m_d")[:])
        nc.gpsimd.dma_start(tiles["queueT8"][:], g("queueT_d")[:])

        # ========== k branch ==========
        _dense_branch(nc, tc, env, cst, 1, pools, tiles, w1tiles=w1k)

        # AG-output loads (wait on the collective, nothing else on sync).
        # Pooled features land in stride-16 staging (img innermost); the
        # big qall block is split per rank so the first logits tiles can
        # start as soon as the head of the buffer lands.
        agv = ag_out[:].rearrange("(r c p) -> c r p", r=N_CORES, c=128)
        for br2, off in ((0, HW), (1, HW + 8)):
            for t in range(CT):
                nc.sync.dma_start(
                    tiles[f"gq_all{br2}"][:, t * 16:t * 16 + 8],
                    agv[:, :, off + t])
        for r in range(N_CORES):
            nc.sync.dma_start(
                tiles["qall"][:, r * HW:(r + 1) * HW], agv[:, r, 0:HW])

        # ========== correspondence (own image, bf16) ==========
        qdT_bf, kdT_bf = tiles["qdT_bf"], tiles["kdT_bf"]
        with tc.tile_pool(name="cor", bufs=1) as cor, \
             tc.tile_pool(name="cor2", bufs=2) as cor2, \
             tc.tile_pool(name="ktp", bufs=7) as ktp:
            # sim + argmax pipelined per partition-tile, with the DVE
            # reading the PSUM sims directly (no SBUF copy)
            mx8 = cor.tile([128, 8], F32, name="mx8")
            ix8 = cor.tile([128, 8], mybir.dt.uint32, name="ix8")
            ixf = cor.tile([128, 7], F32, name="ixf")
            for i, (po, pn) in enumerate(PT):
                s_ps = pools["ps_h"].tile([128, HW], F32, name="s_ps",
                                          tag="h")
                for (o, n) in CH:
                    nc.tensor.matmul(s_ps[0:pn, o:o + n],
                                     lhsT=qdT_bf[:, po:po + pn],
                                     rhs=kdT_bf[:, o:o + n],
                                     start=True, stop=True)
                nc.vector.max(mx8[0:pn, :], s_ps[0:pn, :])
                nc.vector.max_index(ix8[0:pn, :], mx8[0:pn, :],
                                    s_ps[0:pn, :])
                nc.vector.tensor_copy(ixf[0:pn, i:i + 1], ix8[0:pn, 0:1])
            # transposed kd blocks for the gather, hoisted here so the PE
            # has work during the argmax window
            kt_list = []
            for i, (po, pn) in enumerate(PT):
                kt_ps = pools["ps_h"].tile([128, 128], BF16, name="kt_ps",
                                           tag="h")
                nc.tensor.transpose(kt_ps[0:pn, :], kdT_bf[:, po:po + pn],
                                    id_b[:, :])
                kt_sb = ktp.tile([128, 128], BF16, name="kt_sb")
                nc.scalar.activation(kt_sb[0:pn, :], kt_ps[0:pn, :], AF.Copy)
                kt_list.append(kt_sb)

            # broadcast the per-pixel argmax indices down the partitions
            ir_sb = cor.tile([1, HW], F32, name="ir_sb")
            for i, (po, pn) in enumerate(PT):
                ir_ps = pools["ps_m"].tile([1, 128], F32, name="ir_ps",
                                           tag="m")
                nc.tensor.transpose(ir_ps[0:1, 0:pn], ixf[0:pn, i:i + 1],
                                    id_f[0:pn, 0:pn])
                nc.scalar.activation(ir_sb[0:1, po:po + pn],
                                     ir_ps[0:1, 0:pn], AF.Copy)
            ib_ps = pools["ps_m"].tile([128, HW], F32, name="ib_ps", tag="m")
            for (o, n) in CH:
                nc.tensor.matmul(ib_ps[:, o:o + n], lhsT=tiles["onesr"][:],
                                 rhs=ir_sb[:, o:o + n], start=True, stop=True)
            ib_sb = cor.tile([128, HW], F32, name="ib_sb")
            nc.scalar.activation(ib_sb[:], ib_ps[:], AF.Copy)
            # gather matched keys via one-hot matmuls
            mt_ps = pools["ps_m"].tile([128, HW], F32, name="mt_ps", tag="m")
            for i, (po, pn) in enumerate(PT):
                S = cor2.tile([128, HW], BF16, name="S")
                nc.vector.tensor_scalar(
                    S[0:pn, :], ib_sb[0:pn, :], tiles["iota"][0:pn, :],
                    float(po), op0=ALU.subtract, op1=ALU.is_equal)
                for (o, n) in CH:
                    nc.tensor.matmul(mt_ps[:, o:o + n],
                                     lhsT=kt_list[i][0:pn, :],
                                     rhs=S[0:pn, o:o + n],
                                     start=(i == 0), stop=(i == 6))
            nc.scalar.activation(tiles["matchT"][:], mt_ps[:], AF.Copy)
            nc.vector.tensor_scalar_mul(tiles["matchT8"][:], mt_ps[:], 8.0)

            # positives: diag = qd . matched (own rows), summed
            posm = cor.tile([128, HW], F32, name="posm")
            nc.vector.tensor_mul(posm[:], qdT_bf[:], tiles["matchT"][:])
            pos_ps = pools["ps_m"].tile([1, HW], F32, name="pos_ps", tag="m")
            for (o, n) in CH:
                nc.tensor.matmul(pos_ps[:, o:o + n], lhsT=tiles["onesc"][:],
                                 rhs=posm[:, o:o + n], start=True, stop=True)
            nc.vector.reduce_sum(tiles["fin"][0:1, 0:1], pos_ps[:],
                                 axis=mybir.AxisListType.X)

        # ========== gathered q: dense logits (ACT-bound tail) ==========
        with tc.tile_pool(name="escr", bufs=3) as escr:
            # dense logits, column shard: all 6272 q rows x own 784 keys;
            # per-row exp sums via DVE (keeps the ACT chain pure Exp).
            # Everything that depends on the AllGather but not on the
            # correspondence — global heads, lpos, queue negatives — is
            # interleaved into this loop so its latency chains resolve
            # under the exp stream.
            qe_sb = tiles["qe_sb"] = cst.tile([128, 512], BF16, name="qe_sb")

            def qe_group(grp):
                qe_ps = pools["ps_m"].tile([128, 64], F32, name="qe_ps",
                                           tag="m")
                for j in range(8):
                    qt = grp * 8 + j
                    nc.tensor.matmul(
                        qe_ps[:, j * 8:(j + 1) * 8],
                        lhsT=tiles["queueT8"][:, qt * 128:(qt + 1) * 128],
                        rhs=tiles["qg8all"][:], start=(j == 0), stop=(j == 7))
                nc.scalar.activation(qe_sb[:, grp * 64:(grp + 1) * 64],
                                     qe_ps[:], AF.Exp, scale=ISC / 64.0)

            for t in range(RT):
                lg_ps = pools["ps_h"].tile([128, HW], F32, name="lg_ps",
                                           tag="h")
                for (o, n) in CH:
                    nc.tensor.matmul(
                        lg_ps[:, o:o + n],
                        lhsT=tiles["qall"][:, t * 128:(t + 1) * 128],
                        rhs=tiles["matchT8"][:, o:o + n],
                        start=True, stop=True)
                es = escr.tile([128, HW], BF16, name="es")
                nc.scalar.activation(es[:], lg_ps[:], AF.Exp,
                                     scale=ISC / 64.0)
                nc.vector.reduce_sum(tiles["zpart"][:, t:t + 1], es[:],
                                     axis=mybir.AxisListType.X)
                if t == 2:
                    _ghead_all(nc, tc, env, cst, 0, pools, tiles)
                    nc.vector.tensor_scalar_mul(tiles["qg8all"][:],
                                                tiles["qgT_bf"][:], 8.0)
                elif t == 8:
                    _ghead_all(nc, tc, env, cst, 1, pools, tiles)
                    lpm = pools["gp"].tile([128, 8], F32, name="lpm")
                    nc.vector.tensor_mul(lpm[:], tiles["qgT_bf"][:],
                                         tiles["kgT_bf"][:])
                    lp_ps = pools["ps_m"].tile([1, 8], F32, name="lp_ps",
                                               tag="m")
                    nc.tensor.matmul(lp_ps[:], lhsT=tiles["onesc"][:],
                                     rhs=lpm[:], start=True, stop=True)
                    nc.vector.tensor_copy(tiles["fin"][0:1, 1:9], lp_ps[:])
                elif t >= 14 and (t - 14) % 4 == 0 and (t - 14) // 4 < 8:
                    qe_group((t - 14) // 4)
            qs_ps = pools["ps_m"].tile([1, 512], F32, name="qs_ps", tag="m")
            nc.tensor.matmul(qs_ps[:], lhsT=tiles["onescb"][:],
                             rhs=tiles["qe_sb"][:], start=True, stop=True)
            nc.vector.reduce_sum(tiles["fin"][0:1, 9:17],
                                 qs_ps[:].rearrange("p (t i) -> p i t", i=8),
                                 axis=mybir.AxisListType.X)

        nc.sync.dma_start(g("outz_d")[:], tiles["zpart"][:])
        nc.sync.dma_start(g("outs_d")[:], tiles["fin"][:])


def _prep_inputs(inputs):
    fq = np.asarray(inputs["feat_q"], np.float32).reshape(B, HW, C)
    fk = np.asarray(inputs["feat_k"], np.float32).reshape(B, HW, C)

    def xT8(x):  # (784, 1024) -> (128, 8*784) f8 with [c, ct*784+p]
        return np.ascontiguousarray(
            x.reshape(HW, CT, 128).transpose(2, 1, 0).reshape(128, CT * HW)
        ).astype(F8np)

    def w1tile(w):  # (1024, 2048) -> (8, 128, 2048) f8 x64, dt-pair major
        t = (w * 64.0).reshape(CT, 128, DT, 128).transpose(2, 1, 0, 3)
        t = t.reshape(DT // 2, 2, 128, C).transpose(0, 2, 1, 3)
        return np.ascontiguousarray(t.reshape(DT // 2, 128, 2 * C)
                                    ).astype(F8np)

    def w2tile(w):  # (2048, 128) -> (128, 2048) f8 x64
        return np.ascontiguousarray(
            (w * 64.0).reshape(DT, 128, 128).transpose(1, 0, 2)
            .reshape(128, D)).astype(F8np)

    def wg1tile(w):  # (1024, 2048) -> (128, 8*2048) f8 x64
        return np.ascontiguousarray(
            (w * 64.0).reshape(CT, 128, D).transpose(1, 0, 2)
            .reshape(128, CT * D)).astype(F8np)

    shared = {
        "wd1": w1tile(inputs["Wd1"]), "wd1m": w1tile(inputs["mWd1"]),
        "wd2": w2tile(inputs["Wd2"]), "wd2m": w2tile(inputs["mWd2"]),
        "wg1": wg1tile(inputs["Wg1"]), "wg1m": wg1tile(inputs["mWg1"]),
        "wg2": w2tile(inputs["Wg2"]), "wg2m": w2tile(inputs["mWg2"]),
        "bd1": np.ascontiguousarray(
            np.asarray(inputs["bd1"], np.float32).reshape(DT, 128).T),
        "bd1m": np.ascontiguousarray(
            np.asarray(inputs["mbd1"], np.float32).reshape(DT, 128).T),
        "bd2": np.asarray(inputs["bd2"], np.float32).reshape(128, 1),
        "bd2m": np.asarray(inputs["mbd2"], np.float32).reshape(128, 1),
        "bg1r": (np.asarray(inputs["bg1"], np.float32) * 4096.0
                 ).reshape(1, D).astype(BFnp),
        "bg1mr": (np.asarray(inputs["mbg1"], np.float32) * 4096.0
                  ).reshape(1, D).astype(BFnp),
        "bg2": np.asarray(inputs["bg2"], np.float32).reshape(128, 1),
        "bg2m": np.asarray(inputs["mbg2"], np.float32).reshape(128, 1),
        "iota": np.arange(128, dtype=np.float32).reshape(128, 1),
        "onesc": np.ones((128, 1), np.float32),
        "onesr": np.ones((1, 128), np.float32),
        "ones8": np.ones((1, 8), np.float32).astype(BFnp),
    }
    queue = np.asarray(inputs["queue"], np.float32)
    in_maps = []
    for c in range(N_CORES):
        m = dict(shared)
        m["xq"] = xT8(fq[c])
        m["xk"] = xT8(fk[c])
        m["queueT"] = np.ascontiguousarray(
            (queue[c * QSH:(c + 1) * QSH] * 8.0).T).astype(F8np)
        in_maps.append(m)
    return in_maps


_NC = None


def _get_nc():
    global _NC
    if _NC is None:
        _NC = _build()
    return _NC


def _host_combine(outz, outs):
    """outz: [8][128, 49] z-partials; outs: [8][1, 24] scalars.

    outs slots: [0] sum(qd.matched) over own rows, [1:9] lpos per image
    (replicated on every core), [9:17] partial sum(exp(l_neg/tau)) per
    image over the core's queue shard.  Dense z row r=t*128+p lives at
    outz[:, p, t].
    """
    outz = np.asarray(outz, np.float64)   # [8, 128, 49]
    outs = np.asarray(outs, np.float64)   # [8, 24]
    z = outz.sum(axis=0)                  # [128, 49]
    zrows = z.T.reshape(-1)               # row r = t*128+p
    pos_total = outs[:, 0].sum()
    l_d = (np.log(zrows).sum() - ISC * pos_total) / NT
    zq = outs[:, 9:17].sum(axis=0)        # [8]
    lpos = outs[0, 1:9]                   # replicated
    lse = np.log(zq + np.exp(ISC * lpos))
    l_g = np.mean(lse - ISC * lpos)
    return np.float32((1.0 - LAM) * l_g + LAM * l_d).reshape(())


def kernel(**inputs) -> np.ndarray:
    nc = _get_nc()
    in_maps = _prep_inputs(inputs)
    res = bass_utils.run_bass_kernel_spmd(nc, in_maps,
                                          core_ids=list(range(N_CORES)))
    outz = np.stack([res.results[c]["outz"] for c in range(N_CORES)])
    outs = np.stack([res.results[c]["outs"].reshape(24)
                     for c in range(N_CORES)])
    return _host_combine(outz, outs)
